# revision 1
# baseline (speedup 1.0000x reference)
"""Trainium2 Bass kernel for nn_Block_17540646437178 (dense transformer block).

Sharding: data-parallel over B=16 across 8 NeuronCores (2 samples/core,
zero collectives). All matmuls run in bf16 with f32 PSUM accumulation.

Host-side folding (exact, f32): layernorm affines fold into the following
matmul weights/biases; the attention scale folds into W_q; gamma_1/gamma_2
fold into w_proj/fc2; the proj bias folds into a pre-biased residual copy
of x ("xb"); all remaining small biases ship as one packed [128, X] tile.

Attention layout: scores are computed TRANSPOSED (k-tokens on partitions)
so (a) the key-padding mask becomes a per-partition Exp bias, (b) softmax
needs no max-subtraction (logits are O(1); masked lanes underflow to 0),
(c) exp(s + rpb + mask) = exp(s + mask) * exp(rpb) with exp(rpb)
precomputed on host, making the rpb contribution a cheap bf16 multiply
split across DVE and GpSimd. V carries an appended ones-column so the
softmax denominator falls out of the attn@V matmul (column 64 of each
head's 65-wide block), landing per-partition for the normalize multiply.

The text/img FFN split (tokens 0:40 vs 40:616) is handled by DMA-repacking
the post-attention residual into [80, C] and [1152 = 9x128, C] buffers so
every FFN matmul is 128-aligned. DMA *instruction count* on the HWDGE
queues is minimized (each costs ~0.6us serially); latency-insensitive
DMAs (repack, residual reloads, output stores) run on the GpSimd SWDGE.
"""

import numpy as np
import ml_dtypes

BF16NP = ml_dtypes.bfloat16

B, N, C, H, D = 16, 616, 768, 12, 64
TXT = 40
DFF = 3072
NCORES = 8
S = B // NCORES          # 2 samples per core
EPS = 1e-5
SCALE = D ** -0.5
KC = C // 128            # 6 k-tiles over C
MQK = (2 * C) // 128     # 12 m-tiles over q+k features
KF = DFF // 128          # 24 k-tiles over dff
NT = 5                   # token tiles per sample (616 = 4*128 + 104)
TOK_TILES = [(0, 128), (128, 128), (256, 128), (384, 128), (512, 104)]
Q_CHUNKS = [(0, 512), (512, 104)]    # 616 free-dim chunks
C_CHUNKS = [(0, 512), (512, 256)]    # 768 free-dim chunks
IMG = N - TXT            # 576
IMGTOK = S * IMG         # 1152 = 9*128
TXTTOK = S * TXT         # 80
IMG_CHUNK = 384          # img token chunk for FFN (3 chunks)
NEG = -30000.0


def _slab_kp(wt):
    """[K, M] (K = KT*128) -> [128, KT, M] slab layout (partition-major)."""
    k, m = wt.shape
    kt = k // 128
    assert kt * 128 == k
    return np.ascontiguousarray(wt.reshape(kt, 128, m).transpose(1, 0, 2))


def _bf(a):
    return np.ascontiguousarray(a.astype(np.float32)).astype(BF16NP)


def _f32(a):
    return np.ascontiguousarray(np.asarray(a, dtype=np.float32))


def _bcast128(v):
    return np.ascontiguousarray(np.broadcast_to(v.astype(np.float32), (128, v.shape[0])))


def _colmajor(v, nt):
    """(nt*128,) -> [128, nt] with column t holding partitions of tile t."""
    return np.ascontiguousarray(v.astype(np.float32).reshape(nt, 128).T)


def host_prep(inputs):
    """Fold affines/scales into weights; build slab/broadcast layouts.

    Returns (shared, per_core) where per_core is a list of dicts.
    """
    inp = {k: _f32(v) if np.asarray(v).dtype != np.int32 else np.asarray(v)
           for k, v in inputs.items()}

    g1, g2 = inp["gamma_1"], inp["gamma_2"]

    # --- attention: fold ln1 affine + SCALE into w_qkv ---
    wqkv = inp["w_qkv"] * inp["ln1_g"][None, :]
    qkv_b = np.concatenate([inp["q_bias"],
                            np.zeros_like(inp["v_bias"]),
                            inp["v_bias"]])
    qkv_b = qkv_b + inp["w_qkv"] @ inp["ln1_b"]
    wqkv[:C] *= SCALE
    qkv_b[:C] *= SCALE

    w_qk = _slab_kp(_bf(wqkv[: 2 * C].T))            # [128, 6, 1536] bf16
    w_v = _slab_kp(_bf(wqkv[2 * C:].T))              # [128, 6, 768] bf16
    qk_bias = _colmajor(qkv_b[: 2 * C], MQK)         # [128, 12] f32
    v_bias = _bcast128(qkv_b[2 * C:])                # [128, 768] f32

    # --- proj: fold gamma_1 ---
    wpj = g1[:, None] * inp["w_proj"]
    w_pj = _slab_kp(_bf(wpj.T))                      # [128, 6, 768] bf16
    b_pj = _bcast128(g1 * inp["b_proj"])             # [128, 768] f32

    # --- FFN branches: fold ln2 affine into fc1, gamma_2 into fc2 ---
    def ffn(w1, b1, w2, b2, lg, lb):
        w1e = w1 * lg[None, :]
        b1e = b1 + w1 @ lb
        w2e = g2[:, None] * w2
        b2e = g2 * b2
        return w1e, b1e, w2e, b2e

    w1t, b1t, w2t, b2t = ffn(inp["fc1t_w"], inp["fc1t_b"], inp["fc2t_w"],
                             inp["fc2t_b"], inp["ln2t_g"], inp["ln2t_b"])
    w1i, b1i, w2i, b2i = ffn(inp["fc1i_w"], inp["fc1i_b"], inp["fc2i_w"],
                             inp["fc2i_b"], inp["ln2i_g"], inp["ln2i_b"])

    # text fc1 weights grouped by M-slab for streaming: [24, 128, 6, 128]
    w1t_T = _bf(w1t.T)                               # [768, 3072]
    w1t_m = np.ascontiguousarray(
        w1t_T.reshape(KC, 128, KF, 128).transpose(2, 1, 0, 3))
    w2t_k = np.ascontiguousarray(_bf(w2t.T).reshape(KF, 128, C))  # [24,128,768]
    w1i_s = _slab_kp(_bf(w1i.T))                     # [128, 6, 3072]
    w2i_s = _slab_kp(_bf(w2i.T))                     # [128, 24, 768]
    b1t_c = _colmajor(b1t, KF)                       # [128, 24]
    b1i_c = _colmajor(b1i, KF)
    b2t_b = _bcast128(b2t)                           # [128, 768]
    b2i_b = _bcast128(b2i)

    # --- exp(rpb) transposed + k-padded slabs: [12, 128, 5, 616] bf16.
    # Softmax uses exp(s + rpb + maskb) = exp(s + maskb) * exp(rpb); the
    # multiply runs in bf16 on DVE/POOL instead of an f32 PSUM add on DVE.
    rpbT = np.transpose(inp["relative_position_bias"], (0, 2, 1))  # [H, k, q]
    rpb_pad = np.zeros((H, NT * 128, N), np.float32)
    rpb_pad[:, :N, :] = np.exp(rpbT)
    rpb_slab = _bf(np.ascontiguousarray(
        rpb_pad.reshape(H, NT, 128, N).transpose(0, 2, 1, 3)))

    bias_pack = np.concatenate(
        [qk_bias, b1t_c, b1i_c, v_bias, b2t_b, b2i_b], axis=1)
    shared = dict(w_qk=w_qk, w_v=w_v, bias_pack=np.ascontiguousarray(bias_pack),
                  w_pj=w_pj, rpb=rpb_slab, w1t=w1t_m, w2t=w2t_k,
                  w1i=w1i_s, w2i=w2i_s)

    # --- per-core: x shard + mask bias ---
    mask = np.asarray(inputs["mask"]).astype(np.float32)   # [B, N] 0/1
    mb_full = (1.0 - mask) * NEG                            # [B, N]
    mb_pad = np.full((B, NT * 128), NEG, np.float32)
    mb_pad[:, :N] = mb_full
    # xb = x with the (gamma_1-folded) proj bias pre-added: the proj
    # residual eviction then needs a single tensor_add.
    xb_full = inp["x"] + (g1 * inp["b_proj"])[None, None, :]
    per_core = []
    for c in range(NCORES):
        xs = np.ascontiguousarray(inp["x"][c * S:(c + 1) * S])
        xbs = np.ascontiguousarray(xb_full[c * S:(c + 1) * S]).astype(np.float32)
        mb = np.ascontiguousarray(
            mb_pad[c * S:(c + 1) * S].reshape(S, NT, 128).transpose(0, 2, 1))
        per_core.append(dict(x=xs, xb=xbs, maskb=mb))
    return shared, per_core


def build_program(ablate=None):
    """Build the per-core Bass/Tile program. Returns compiled nc.

    ablate: None/"full", or one of "ln","qkv","attn","proj" to stop
    emission after that phase (timing experiments only — output garbage).
    """
    import os
    if ablate is None:
        ablate = os.environ.get("KERNEL_ABLATE", "full")
    LVL = {"ln": 1, "qkv": 2, "attn": 3, "proj": 4, "full": 9}[ablate]
    off = set(os.environ.get("KERNEL_OFF", "").split(","))
    # tensor_tensor_reduce is a custom DVE ISA op whose ucode is not loaded
    # on this deployment — using it hangs the device. Permanently off.
    USE_TTR = False
    POOL_MUL = "poolmul" not in off   # exp*erpb multiplies on GpSimd
    POOL_DMA = "pooldma" not in off   # late DMAs on GpSimd SWDGE
    REPS = int(os.environ.get("KERNEL_REPS", "1"))
    from contextlib import ExitStack
    import concourse.bass as bass
    import concourse.mybir as mybir
    import concourse.tile as tile
    from concourse import bacc
    from concourse.masks import make_identity

    f32 = mybir.dt.float32
    bf16 = mybir.dt.bfloat16
    Af = mybir.ActivationFunctionType
    Ax = mybir.AxisListType
    Op = mybir.AluOpType

    nc = bacc.Bacc("TRN2", target_bir_lowering=False, debug=False,
                   num_devices=NCORES)

    x_d = nc.declare_dram_parameter("x", [S, N, C], f32, isOutput=False)
    xb_d = nc.declare_dram_parameter("xb", [S, N, C], f32, isOutput=False)
    mb_d = nc.declare_dram_parameter("maskb", [S, 128, NT], f32, isOutput=False)
    wqk_d = nc.declare_dram_parameter("w_qk", [128, KC, 2 * C], bf16, isOutput=False)
    wv_d = nc.declare_dram_parameter("w_v", [128, KC, C], bf16, isOutput=False)
    bp_d = nc.declare_dram_parameter("bias_pack",
                                     [128, MQK + 2 * KF + 3 * C], f32,
                                     isOutput=False)
    wpj_d = nc.declare_dram_parameter("w_pj", [128, KC, C], bf16, isOutput=False)
    rpb_d = nc.declare_dram_parameter("rpb", [H, 128, NT, N], bf16, isOutput=False)
    w1t_d = nc.declare_dram_parameter("w1t", [KF, 128, KC, 128], bf16, isOutput=False)
    w2t_d = nc.declare_dram_parameter("w2t", [KF, 128, C], bf16, isOutput=False)
    w1i_d = nc.declare_dram_parameter("w1i", [128, KC, DFF], bf16, isOutput=False)
    w2i_d = nc.declare_dram_parameter("w2i", [128, KF, C], bf16, isOutput=False)
    out_d = nc.declare_dram_parameter("out", [S, N, C], f32, isOutput=True)

    with tile.TileContext(nc, pool_alloc_mode="queue") as tc, \
            ExitStack() as ctx:
        # ---------- pools ----------
        pers = ctx.enter_context(tc.tile_pool(name="pers", bufs=1))
        psum = ctx.enter_context(tc.tile_pool(name="psum", bufs=1, space="PSUM"))

        def ps_tile(name, wide):
            if wide > 256:
                return psum.tile([128, 512], f32, name=name, tag="big", bufs=3)
            return psum.tile([128, 256], f32, name=name, tag="sm", bufs=1)

        # ---------- persistent constants ----------
        ident = pers.tile([128, 128], bf16, name="ident")
        make_identity(nc, ident)
        bias_pack = pers.tile([128, MQK + 2 * KF + 3 * C], f32,
                              name="bias_pack")
        qkb = bias_pack[:, 0:MQK]
        b1t = bias_pack[:, MQK:MQK + KF]
        b1i = bias_pack[:, MQK + KF:MQK + 2 * KF]
        vb = bias_pack[:, MQK + 2 * KF:MQK + 2 * KF + C]
        b2t = bias_pack[:, MQK + 2 * KF + C:MQK + 2 * KF + 2 * C]
        b2i = bias_pack[:, MQK + 2 * KF + 2 * C:MQK + 2 * KF + 3 * C]
        mb = pers.tile([128, S, NT], f32, name="mb")
        x2rep_img = pers.tile([128, 9, C], f32, name="x2rep_img")
        x2rep_txt = pers.tile([128, C], f32, name="x2rep_txt")
        eps_t = pers.tile([128, 1], f32, name="eps_t")
        nc.vector.memset(eps_t[:], EPS)

        # ---------- helpers ----------
        def layer_norm(pool, src_ap, tp, dst_ap):
            """dst(bf16) = (src - mean)/sqrt(var+EPS); src [tp, C] f32."""
            sm = pool.tile([128, 1], f32, name="ln_sm", tag="ln_sm", bufs=4)
            nc.vector.tensor_reduce(sm[0:tp], src_ap, Ax.X, Op.add)
            nm = pool.tile([128, 1], f32, name="ln_nm", tag="ln_nm", bufs=4)
            nc.scalar.mul(nm[0:tp], sm[0:tp], -1.0 / C)
            xc = pool.tile([128, C], f32, name="ln_xc", tag="ln_xc", bufs=2)
            nc.vector.tensor_scalar_add(xc[0:tp], src_ap, nm[0:tp])
            sq = pool.tile([128, C], f32, name="ln_sq", tag="ln_sq", bufs=2)
            ssq = pool.tile([128, 1], f32, name="ln_ssq", tag="ln_ssq", bufs=4)
            if USE_TTR:
                nc.vector.tensor_tensor_reduce(
                    sq[0:tp], xc[0:tp], xc[0:tp], 1.0, 0.0,
                    Op.mult, Op.add, ssq[0:tp])
            else:
                nc.scalar.activation(sq[0:tp], xc[0:tp], Af.Square,
                                     accum_out=ssq[0:tp])
            std = pool.tile([128, 1], f32, name="ln_std", tag="ln_std", bufs=4)
            nc.scalar.activation(std[0:tp], ssq[0:tp], Af.Sqrt,
                                 bias=eps_t[0:tp], scale=1.0 / C)
            rstd = pool.tile([128, 1], f32, name="ln_rstd", tag="ln_rstd", bufs=4)
            nc.vector.reciprocal(rstd[0:tp], std[0:tp])
            nc.vector.tensor_scalar_mul(dst_ap, xc[0:tp], rstd[0:tp])

        def late_dma(out_ap, in_ap):
            (nc.gpsimd if POOL_DMA else nc.sync).dma_start(out_ap, in_ap)

        tp_flip = [0]

        def transpose_block(src_full_ap, dst_full_ap):
            """dst[128,128] = src[128,128].T via PE; evictions alternate
            ACT/DVE to balance engine load. Rows beyond the valid token
            count carry garbage into padded dst columns (never read)."""
            ps = psum.tile([128, 128], bf16, name="tps", tag="tp", bufs=2)
            nc.tensor.transpose(ps[:], src_full_ap, ident[:])
            tp_flip[0] ^= 1
            if tp_flip[0]:
                nc.scalar.copy(dst_full_ap, ps[:])
            else:
                nc.vector.tensor_copy(dst_full_ap, ps[:])

        for _rep in range(REPS):
            # ================= attention era =================
            with tc.tile_pool(name="era", bufs=1) as era:
                xT = {}
                qkT = {}
                vsb = {}
                osb = {}
                x2 = {}

                with tc.tile_pool(name="wqkv", bufs=1) as wp:
                    wqk = wp.tile([128, KC, 2 * C], bf16, name="wqk")
                    wv = wp.tile([128, KC, C], bf16, name="wv")

                    # ---- LN1 + transpose to xT ----
                    for s in range(S):
                        xT[s] = era.tile([128, KC, 640], bf16, name=f"xT{s}",
                                         tag="xT", bufs=2)
                        for (t0, tp) in TOK_TILES:
                            xin = era.tile([128, C], f32, name="xin", tag="xin",
                                           bufs=4)
                            nc.sync.dma_start(xin[0:tp], x_d[s, t0:t0 + tp, :])
                            xh = era.tile([128, C], bf16, name="xh", tag="xh",
                                          bufs=3)
                            if tp < 128:
                                nc.vector.memset(xh[96:128, :], 0.0)
                            layer_norm(era, xin[0:tp], tp, xh[0:tp])
                            for f in range(KC):
                                transpose_block(xh[:, f * 128:(f + 1) * 128],
                                                xT[s][:, f, t0:t0 + 128])

                    if _rep == 0:
                        nc.sync.dma_start(bias_pack[:], bp_d[:])
                        nc.sync.dma_start(mb[:],
                                          mb_d[:].rearrange("s p t -> p s t"))
                    nc.sync.dma_start(wqk[:], wqk_d[:])
                    nc.sync.dma_start(wv[:], wv_d[:])
                    # ---- QKV projections ----
                    for s in range(S if LVL >= 2 else 0):
                        qkT[s] = era.tile([128, MQK, N], bf16, name=f"qkT{s}",
                                          tag="qkT", bufs=2)
                        for m in range(MQK):
                            for (q0, qn) in Q_CHUNKS:
                                ps = ps_tile("ps_qk", qn)
                                for k in range(KC):
                                    nc.tensor.matmul(
                                        ps[:, 0:qn],
                                        wqk[:, k, m * 128:(m + 1) * 128],
                                        xT[s][:, k, q0:q0 + qn],
                                        start=(k == 0), stop=(k == KC - 1))
                                nc.vector.tensor_scalar_add(
                                    qkT[s][:, m, q0:q0 + qn], ps[:, 0:qn],
                                    qkb[:, m:m + 1])
                        vsb[s] = era.tile([128, NT, H * 65], bf16, name=f"v{s}",
                                          tag="v", bufs=2)
                        for ti, (t0, tp) in enumerate(TOK_TILES):
                            for (n0, nn) in C_CHUNKS:
                                ps = ps_tile("ps_v", nn)
                                for k in range(KC):
                                    nc.tensor.matmul(
                                        ps[0:tp, 0:nn],
                                        xT[s][:, k, t0:t0 + tp],
                                        wv[:, k, n0:n0 + nn],
                                        start=(k == 0), stop=(k == KC - 1))
                                nh = nn // 64
                                h0 = n0 // 64
                                vview = vsb[s][0:tp, ti, :].rearrange(
                                    "p (h e) -> p h e", e=65)[:, h0:h0 + nh, 0:64]
                                nc.vector.tensor_add(
                                    vview,
                                    ps[0:tp, 0:nn].rearrange("p (h e) -> p h e",
                                                             e=64),
                                    vb[0:tp, n0:n0 + nn].rearrange(
                                        "p (h e) -> p h e", e=64))
                            ones = vsb[s][0:tp, ti, :].rearrange(
                                "p (h e) -> p h e", e=65)[:, :, 64:65]
                            nc.vector.memset(ones, 1.0)

                # ---- attention core ----
                for s in range(S if LVL >= 3 else 0):
                    osb[s] = era.tile([128, NT, C], bf16, name=f"o{s}",
                                      tag="o", bufs=2)
                    nc.vector.memset(osb[s][96:128, NT - 1, :], 0.0)
                with tc.tile_pool(name="attn", bufs=1) as apool:
                    for s in range(S if LVL >= 3 else 0):
                        for h in range(H):
                            rpb = apool.tile([128, NT, N], bf16, name="rpb",
                                             tag="rpb", bufs=2)
                            nc.sync.dma_start(rpb[:], rpb_d[h])
                            mtile = KC + h // 2
                            qtile = h // 2
                            base = (h % 2) * 64
                            expT = apool.tile([128, NT, N], bf16, name="expT",
                                              tag="expT", bufs=2)
                            for kt, (k0, tp) in enumerate(TOK_TILES):
                                eraw = apool.tile([128, N], bf16, name="eraw",
                                                  tag="eraw", bufs=4)
                                for (q0, qn) in Q_CHUNKS:
                                    ps = ps_tile("ps_sc", qn)
                                    nc.tensor.matmul(
                                        ps[0:tp, 0:qn],
                                        qkT[s][base:base + 64, mtile, k0:k0 + tp],
                                        qkT[s][base:base + 64, qtile, q0:q0 + qn],
                                        start=True, stop=True)
                                    nc.scalar.activation(
                                        eraw[0:tp, q0:q0 + qn],
                                        ps[0:tp, 0:qn], Af.Exp,
                                        bias=mb[0:tp, s, kt:kt + 1])
                                eng = (nc.gpsimd if (POOL_MUL and kt % 3 == 2)
                                       else nc.vector)
                                eng.tensor_mul(expT[0:tp, kt, :],
                                               eraw[0:tp, :],
                                               rpb[0:tp, kt, :])
                            for qt, (qq0, qp) in enumerate(TOK_TILES):
                                ops = psum.tile([128, 65], f32, name="ops",
                                                tag="tiny", bufs=2)
                                for kt, (k0, tp) in enumerate(TOK_TILES):
                                    nc.tensor.matmul(
                                        ops[0:qp, :],
                                        expT[0:tp, kt, qq0:qq0 + qp],
                                        vsb[s][0:tp, kt, h * 65:(h + 1) * 65],
                                        start=(kt == 0), stop=(kt == NT - 1))
                                rc = era.tile([128, 1], f32, name="rc", tag="rc",
                                              bufs=4)
                                nc.vector.reciprocal(rc[0:qp], ops[0:qp, 64:65])
                                nc.vector.tensor_scalar_mul(
                                    osb[s][0:qp, qt, h * 64:(h + 1) * 64],
                                    ops[0:qp, 0:64], rc[0:qp])

                # ---- proj + residual ----
                with tc.tile_pool(name="proj", bufs=1) as pp:
                    wpj = pp.tile([128, KC, C], bf16, name="wpj")
                    nc.sync.dma_start(wpj[:], wpj_d[:])
                    for s in range(S if LVL >= 4 else 0):
                        oT = era.tile([128, KC, 640], bf16, name=f"oT{s}",
                                      tag="xT", bufs=2)
                        for ti, (t0, tp) in enumerate(TOK_TILES):
                            for f in range(KC):
                                transpose_block(
                                    osb[s][:, ti, f * 128:(f + 1) * 128],
                                    oT[:, f, t0:t0 + 128])
                        x2[s] = era.tile([128, NT, C], f32, name=f"x2_{s}",
                                         tag="x2", bufs=2)
                        for ti, (t0, tp) in enumerate(TOK_TILES):
                            xres = pp.tile([128, C], f32, name="xres", tag="xres",
                                           bufs=2)
                            late_dma(xres[0:tp], xb_d[s, t0:t0 + tp, :])
                            for (n0, nn) in C_CHUNKS:
                                ps = ps_tile("ps_pj", nn)
                                for k in range(KC):
                                    nc.tensor.matmul(
                                        ps[0:tp, 0:nn],
                                        oT[:, k, t0:t0 + tp],
                                        wpj[:, k, n0:n0 + nn],
                                        start=(k == 0), stop=(k == KC - 1))
                                nc.vector.tensor_add(
                                    x2[s][0:tp, ti, n0:n0 + nn],
                                    ps[0:tp, 0:nn], xres[0:tp, n0:n0 + nn])

                # ---- repack x2 -> text [80, C] + img [1152 (9x128), C] ----
                for s in range(S if LVL >= 4 else 0):
                    nc.sync.dma_start(x2rep_txt[40 * s:40 * s + 40, :],
                                      x2[s][0:40, 0, :])
                    # img rows: seq 40..616 -> global 576*s ..
                    g = 576 * s
                    for kt, (t0, tp) in enumerate(TOK_TILES):
                        p0 = 40 if kt == 0 else 0
                        length = tp - p0
                        src_off = p0
                        while length > 0:
                            j, dp = g // 128, g % 128
                            piece = min(length, 128 - dp)
                            nc.sync.dma_start(
                                x2rep_img[dp:dp + piece, j, :],
                                x2[s][src_off:src_off + piece, kt, :])
                            g += piece
                            src_off += piece
                            length -= piece

            # ================= FFN era =================
            if LVL >= 5:
                with tc.tile_pool(name="ffn", bufs=1) as fp:
                    w1i = fp.tile([128, KC, DFF], bf16, name="w1i")
                    w2i = fp.tile([128, KF, C], bf16, name="w2i")
                    for k in range(KC):
                        nc.sync.dma_start(w1i[:, k, :], w1i_d[:, k, :])
                    nc.sync.dma_start(w2i[:, 0:12, :], w2i_d[:, 0:12, :])
                    nc.sync.dma_start(w2i[:, 12:24, :], w2i_d[:, 12:24, :])
                    # LN2 + transpose
                    ztT = fp.tile([128, KC, 128], bf16, name="ztT")
                    xh2 = fp.tile([128, C], bf16, name="xh2", tag="xh2", bufs=2)
                    nc.vector.memset(xh2[64:128, :], 0.0)
                    layer_norm(fp, x2rep_txt[0:TXTTOK], TXTTOK, xh2[0:TXTTOK])
                    for f in range(KC):
                        transpose_block(xh2[:, f * 128:(f + 1) * 128],
                                        ztT[:, f, 0:128])
                    ziT = fp.tile([128, KC, IMGTOK], bf16, name="ziT")
                    for j in range(9):
                        xh2 = fp.tile([128, C], bf16, name="xh2", tag="xh2", bufs=2)
                        layer_norm(fp, x2rep_img[:, j, :], 128, xh2[:])
                        for f in range(KC):
                            transpose_block(xh2[:, f * 128:(f + 1) * 128],
                                            ziT[:, f, j * 128:(j + 1) * 128])
                    # Pre-add the (gamma_2-folded) fc2 biases into the residual so
                    # each fc2 eviction is a single tensor_add. In-place; Tile
                    # orders these after the LN2 reads above.
                    nc.vector.tensor_add(x2rep_txt[0:TXTTOK, :], x2rep_txt[0:TXTTOK, :],
                                         b2t[0:TXTTOK, :])
                    for j in range(9):
                        nc.vector.tensor_add(x2rep_img[:, j, :], x2rep_img[:, j, :],
                                             b2i[:, :])

                    # ---- img FFN (resident weights, 3 token chunks) ----
                    for c in range(3):
                        q0 = c * IMG_CHUNK
                        hgi = fp.tile([128, KF, IMG_CHUNK], bf16, name="hgi",
                                      tag="hgi", bufs=1)
                        for m in range(KF):
                            ps = ps_tile("ps_f1i", 512)
                            for k in range(KC):
                                nc.tensor.matmul(ps[:, 0:IMG_CHUNK],
                                                 w1i[:, k, m * 128:(m + 1) * 128],
                                                 ziT[:, k, q0:q0 + IMG_CHUNK],
                                                 start=(k == 0), stop=(k == KC - 1))
                            nc.scalar.activation(hgi[:, m, :], ps[:, 0:IMG_CHUNK],
                                                 Af.Gelu, bias=b1i[:, m:m + 1])
                        for mt in range(3):
                            j = 3 * c + mt
                            ps0 = ps_tile("ps_f2i0", 512)
                            ps1 = ps_tile("ps_f2i1", 256)
                            for k in range(KF):
                                nc.tensor.matmul(ps0[:, 0:512],
                                                 hgi[:, k, mt * 128:(mt + 1) * 128],
                                                 w2i[:, k, 0:512],
                                                 start=(k == 0), stop=(k == KF - 1))
                                nc.tensor.matmul(ps1[:, 0:256],
                                                 hgi[:, k, mt * 128:(mt + 1) * 128],
                                                 w2i[:, k, 512:768],
                                                 start=(k == 0), stop=(k == KF - 1))
                            ot = fp.tile([128, C], f32, name="ot", tag="ost", bufs=3)
                            for (n0, nn), ps in zip(C_CHUNKS, [ps0, ps1]):
                                nc.vector.tensor_add(ot[:, n0:n0 + nn], ps[:, 0:nn],
                                                     x2rep_img[:, j, n0:n0 + nn])
                            # DMA out: global img row g = 128*j -> (b, 40 + g%576)
                            g0 = 128 * j
                            p = 0
                            while p < 128:
                                g = g0 + p
                                b = g // IMG
                                piece = min(128 - p, IMG * (b + 1) - g)
                                late_dma(
                                    out_d[b, TXT + g - b * IMG:
                                          TXT + g - b * IMG + piece, :],
                                    ot[p:p + piece, :])
                                p += piece

                    # ---- text FFN (streamed weights) ----
                    with tc.tile_pool(name="wtxt", bufs=1) as wt:
                        hgt = fp.tile([128, KF, TXTTOK], bf16, name="hgt")
                        for mc in range(8):
                            w1tc = wt.tile([128, 3, KC * 128], bf16,
                                           name="w1tc", tag="w1tc", bufs=2)
                            nc.sync.dma_start(
                                w1tc[:],
                                w1t_d[3 * mc:3 * mc + 3].rearrange(
                                    "m p k n -> p m (k n)"))
                            for ml in range(3):
                                m = 3 * mc + ml
                                ps = ps_tile("ps_f1t", 512)
                                for k in range(KC):
                                    nc.tensor.matmul(
                                        ps[:, 0:TXTTOK],
                                        w1tc[:, ml, k * 128:(k + 1) * 128],
                                        ztT[:, k, 0:TXTTOK],
                                        start=(k == 0), stop=(k == KC - 1))
                                nc.scalar.activation(
                                    hgt[:, m, 0:TXTTOK], ps[:, 0:TXTTOK],
                                    Af.Gelu, bias=b1t[:, m:m + 1])
                        ps0 = ps_tile("ps_f2t0", 512)
                        ps1 = ps_tile("ps_f2t1", 256)
                        for kc4 in range(8):
                            w2tc = wt.tile([128, 3, C], bf16, name="w2tc",
                                           tag="w2tc", bufs=2)
                            nc.sync.dma_start(
                                w2tc[:],
                                w2t_d[3 * kc4:3 * kc4 + 3].rearrange(
                                    "k p n -> p k n"))
                            for kl in range(3):
                                k = 3 * kc4 + kl
                                nc.tensor.matmul(
                                    ps0[0:TXTTOK, 0:512], hgt[:, k, 0:TXTTOK],
                                    w2tc[:, kl, 0:512],
                                    start=(k == 0), stop=(k == KF - 1))
                                nc.tensor.matmul(
                                    ps1[0:TXTTOK, 0:256], hgt[:, k, 0:TXTTOK],
                                    w2tc[:, kl, 512:768],
                                    start=(k == 0), stop=(k == KF - 1))
                        ot = fp.tile([128, C], f32, name="ot", tag="ost", bufs=3)
                        for (n0, nn), ps in zip(C_CHUNKS, [ps0, ps1]):
                            nc.vector.tensor_add(ot[0:TXTTOK, n0:n0 + nn],
                                                 ps[0:TXTTOK, 0:nn],
                                                 x2rep_txt[0:TXTTOK, n0:n0 + nn])
                        for s in range(S):
                            late_dma(out_d[s, 0:TXT, :],
                                     ot[40 * s:40 * s + 40, :])

    nc.compile()
    return nc


_CACHE = {}


def _get_program():
    if "nc" not in _CACHE:
        _CACHE["nc"] = build_program()
    return _CACHE["nc"]


def run(inputs, trace=False):
    from concourse.bass_utils import run_bass_kernel_spmd
    shared, per_core = host_prep(inputs)
    nc = _get_program()
    in_maps = [{**shared, **pc} for pc in per_core]
    res = run_bass_kernel_spmd(nc, in_maps, core_ids=list(range(NCORES)),
                               trace=trace)
    out = np.concatenate([res.results[i]["out"] for i in range(NCORES)],
                         axis=0).astype(np.float32)
    return out, res


def kernel(**inputs):
    out, _ = run(inputs, trace=False)
    return out



# revision 34
# speedup vs baseline: 1.3288x; 1.3288x over previous
"""Trainium2 Bass kernel for nn_Block_17540646437178 (dense transformer block).

Sharding: data-parallel over B=16 across 8 NeuronCores (2 samples/core,
zero collectives).

v2: all large matmuls (QKV, V, proj, both FFNs, attn@V) run in fp8e4m3
with DoubleRow perf mode (256-deep contraction pairs, f32 PSUM accum);
scores (K=64) stay bf16. PE transposes stay bf16 (fp8 transpose is
broken in the toolchain); fp8 conversion is folded into the PSUM
evictions that happen anyway. Attention loops h-outer/s-inner so each
exp(rpb) slab is DMA'd once and reused for both samples.

Host-side folding (exact, f32): layernorm affines fold into the following
matmul weights/biases; the attention scale folds into W_q; gamma_1/gamma_2
fold into w_proj/fc2; the proj bias folds into a pre-biased residual copy
of x ("xb"); all remaining small biases ship as one packed [128, X] tile.

Attention layout: scores are computed TRANSPOSED (k-tokens on partitions)
so (a) the key-padding mask becomes a per-partition Exp bias, (b) softmax
needs no max-subtraction (logits are O(1); masked lanes underflow to 0),
(c) exp(s + rpb + mask) = exp(s + mask) * exp(rpb) with exp(rpb)
precomputed on host (bf16 multiply on DVE, fp8 output). V carries an
appended ones-column so the softmax denominator falls out of the attn@V
matmul, landing per-partition for the normalize multiply.

The text/img FFN split (tokens 0:40 vs 40:616) is handled by DMA-repacking
the post-attention residual into [80, C] and [1152 = 9x128, C] buffers so
every FFN matmul is 128-aligned.
"""

import numpy as np
import ml_dtypes

BF16NP = ml_dtypes.bfloat16
F8NP = ml_dtypes.float8_e4m3

B, N, C, H, D = 16, 616, 768, 12, 64
TXT = 40
DFF = 3072
NCORES = 8
S = B // NCORES          # 2 samples per core
EPS = 1e-5
SCALE = D ** -0.5
KC = C // 128            # 6 k-tiles over C
KP = KC // 2             # 3 DoubleRow k-pairs over C
MQK = (2 * C) // 128     # 12 m-tiles over q+k features
KF = DFF // 128          # 24 k-tiles over dff
KFP = KF // 2            # 12 DoubleRow k-pairs over dff
NT = 5                   # token tiles per sample (616 = 4*128 + 104)
TOK_TILES = [(0, 128), (128, 128), (256, 128), (384, 128), (512, 104)]
Q_CHUNKS = [(0, 512), (512, 104)]    # 616 free-dim chunks
C_CHUNKS = [(0, 512), (512, 256)]    # 768 free-dim chunks
IMG = N - TXT            # 576
IMGTOK = S * IMG         # 1152 = 9*128
TXTTOK = S * TXT         # 80
IMG_CHUNK = 384          # img token chunk for FFN (3 chunks)
NEG = -30000.0
HS = 80                  # vsb per-head stride (64 V + 1 ones + pad, %16==0)
NPAD = 640               # expT row stride (%16==0)


def _slab_kp(wt, np_dt):
    """[K, M] (K = KT*128) -> [128, KT, M] slab layout (partition-major)."""
    k, m = wt.shape
    kt = k // 128
    assert kt * 128 == k
    return np.ascontiguousarray(
        wt.astype(np.float32).reshape(kt, 128, m).transpose(1, 0, 2)).astype(np_dt)


def _bf(a):
    return np.ascontiguousarray(np.asarray(a, np.float32)).astype(BF16NP)


def _f32(a):
    return np.ascontiguousarray(np.asarray(a, dtype=np.float32))


def _bcast128(v):
    return np.ascontiguousarray(np.broadcast_to(v.astype(np.float32), (128, v.shape[0])))


def _colmajor(v, nt):
    """(nt*128,) -> [128, nt] with column t holding partitions of tile t."""
    return np.ascontiguousarray(v.astype(np.float32).reshape(nt, 128).T)


def host_prep(inputs):
    """Fold affines/scales into weights; build slab/broadcast layouts.

    Returns (shared, per_core) where per_core is a list of dicts.
    """
    inp = {k: _f32(v) if np.asarray(v).dtype != np.int32 else np.asarray(v)
           for k, v in inputs.items()}

    g1, g2 = inp["gamma_1"], inp["gamma_2"]

    # --- attention: fold ln1 affine + SCALE into w_qkv ---
    wqkv = inp["w_qkv"] * inp["ln1_g"][None, :]
    qkv_b = np.concatenate([inp["q_bias"],
                            np.zeros_like(inp["v_bias"]),
                            inp["v_bias"]])
    qkv_b = qkv_b + inp["w_qkv"] @ inp["ln1_b"]
    wqkv[:C] *= SCALE
    qkv_b[:C] *= SCALE

    w_qk = _slab_kp(wqkv[: 2 * C].T, F8NP)           # [128, 6, 1536] fp8
    w_v = _slab_kp(wqkv[2 * C:].T, F8NP)             # [128, 6, 768] fp8
    qk_bias = _colmajor(qkv_b[: 2 * C], MQK)         # [128, 12] f32
    v_bias = _bcast128(qkv_b[2 * C:])                # [128, 768] f32

    # --- proj: fold gamma_1 ---
    wpj = g1[:, None] * inp["w_proj"]
    w_pj = _slab_kp(wpj.T, F8NP)                     # [128, 6, 768] fp8
    b_pj = _bcast128(g1 * inp["b_proj"])             # [128, 768] f32

    # --- FFN branches: fold ln2 affine into fc1, gamma_2 into fc2 ---
    def ffn(w1, b1, w2, b2, lg, lb):
        w1e = w1 * lg[None, :]
        b1e = b1 + w1 @ lb
        w2e = g2[:, None] * w2
        b2e = g2 * b2
        return w1e, b1e, w2e, b2e

    w1t, b1t, w2t, b2t = ffn(inp["fc1t_w"], inp["fc1t_b"], inp["fc2t_w"],
                             inp["fc2t_b"], inp["ln2t_g"], inp["ln2t_b"])
    w1i, b1i, w2i, b2i = ffn(inp["fc1i_w"], inp["fc1i_b"], inp["fc2i_w"],
                             inp["fc2i_b"], inp["ln2i_g"], inp["ln2i_b"])

    # text fc1 weights grouped by M-slab for streaming: [24, 128, 6, 128] fp8
    w1t_T = w1t.T.astype(np.float32)                 # [768, 3072]
    w1t_m = np.ascontiguousarray(
        w1t_T.reshape(KC, 128, KF, 128).transpose(2, 1, 0, 3)).astype(F8NP)
    w2t_k = np.ascontiguousarray(
        w2t.T.astype(np.float32).reshape(KF, 128, C)).astype(F8NP)  # [24,128,768]
    w1i_s = _slab_kp(w1i.T, F8NP)                    # [128, 6, 3072]
    w2i_s = _slab_kp(w2i.T, F8NP)                    # [128, 24, 768]
    b1t_c = _colmajor(b1t, KF)                       # [128, 24]
    b1i_c = _colmajor(b1i, KF)
    b2t_b = _bcast128(b2t)                           # [128, 768]
    b2i_b = _bcast128(b2i)

    # --- rpb transposed + k-padded slabs: [12, 128, 5, 616] bf16.
    # rpb is accumulated into the scores PSUM via an identity matmul, so
    # the ACT Exp eviction computes exp(s + rpb + maskb) directly (fp8 out).
    rpbT = np.transpose(inp["relative_position_bias"], (0, 2, 1))  # [H, k, q]
    rpb_pad = np.zeros((H, NT * 128, N), np.float32)
    rpb_pad[:, :N, :] = rpbT
    rpb_slab = np.ascontiguousarray(
        rpb_pad.reshape(H, NT, 128, N).transpose(0, 2, 1, 3)).astype(F8NP)

    bias_pack = np.concatenate(
        [qk_bias, b1t_c, b1i_c, v_bias, b2t_b, b2i_b], axis=1)
    shared = dict(w_qk=w_qk, w_v=w_v, bias_pack=np.ascontiguousarray(bias_pack),
                  w_pj=w_pj, rpb=rpb_slab, w1t=w1t_m, w2t=w2t_k,
                  w1i=w1i_s, w2i=w2i_s)

    # --- per-core: x shard + mask bias ---
    mask = np.asarray(inputs["mask"]).astype(np.float32)   # [B, N] 0/1
    mb_full = (1.0 - mask) * NEG                            # [B, N]
    mb_pad = np.full((B, NT * 128), NEG, np.float32)
    mb_pad[:, :N] = mb_full
    # xb = x with the (gamma_1-folded) proj bias pre-added: the proj
    # residual eviction then needs a single tensor_add.
    xb_full = inp["x"] + (g1 * inp["b_proj"])[None, None, :]
    per_core = []
    for c in range(NCORES):
        xs = np.ascontiguousarray(inp["x"][c * S:(c + 1) * S]).astype(BF16NP)
        xbs = np.ascontiguousarray(xb_full[c * S:(c + 1) * S]).astype(BF16NP)
        mb = np.ascontiguousarray(
            mb_pad[c * S:(c + 1) * S].reshape(S, NT, 128).transpose(0, 2, 1))
        per_core.append(dict(x=xs, xb=xbs, maskb=mb))
    return shared, per_core


def build_program(ablate=None):
    """Build the per-core Bass/Tile program. Returns compiled nc.

    ablate: None/"full", or one of "ln","qkv","attn","proj" to stop
    emission after that phase (timing experiments only — output garbage).
    """
    import os
    if ablate is None:
        ablate = os.environ.get("KERNEL_ABLATE", "full")
    LVL = {"ln": 1, "qkv": 2, "attn": 3, "proj": 4, "full": 9}[ablate]
    off = set(os.environ.get("KERNEL_OFF", "").split(","))
    POOL_DMA = "pooldma" not in off   # late DMAs on GpSimd SWDGE
    REPS = int(os.environ.get("KERNEL_REPS", "1"))
    from contextlib import ExitStack
    import concourse.bass as bass
    import concourse.mybir as mybir
    import concourse.tile as tile
    from concourse import bacc
    from concourse.masks import make_identity

    f32 = mybir.dt.float32
    bf16 = mybir.dt.bfloat16
    fp8 = mybir.dt.float8e4
    DR = mybir.MatmulPerfMode.DoubleRow
    Af = mybir.ActivationFunctionType
    Ax = mybir.AxisListType
    Op = mybir.AluOpType

    nc = bacc.Bacc("TRN2", target_bir_lowering=False, debug=False,
                   num_devices=NCORES)

    x_d = nc.declare_dram_parameter("x", [S, N, C], bf16, isOutput=False)
    xb_d = nc.declare_dram_parameter("xb", [S, N, C], bf16, isOutput=False)
    mb_d = nc.declare_dram_parameter("maskb", [S, 128, NT], f32, isOutput=False)
    wqk_d = nc.declare_dram_parameter("w_qk", [128, KC, 2 * C], fp8, isOutput=False)
    wv_d = nc.declare_dram_parameter("w_v", [128, KC, C], fp8, isOutput=False)
    bp_d = nc.declare_dram_parameter("bias_pack",
                                     [128, MQK + 2 * KF + 3 * C], f32,
                                     isOutput=False)
    wpj_d = nc.declare_dram_parameter("w_pj", [128, KC, C], fp8, isOutput=False)
    rpb_d = nc.declare_dram_parameter("rpb", [H, 128, NT, N], fp8, isOutput=False)
    w1t_d = nc.declare_dram_parameter("w1t", [KF, 128, KC, 128], fp8, isOutput=False)
    w2t_d = nc.declare_dram_parameter("w2t", [KF, 128, C], fp8, isOutput=False)
    w1i_d = nc.declare_dram_parameter("w1i", [128, KC, DFF], fp8, isOutput=False)
    w2i_d = nc.declare_dram_parameter("w2i", [128, KF, C], fp8, isOutput=False)
    out_d = nc.declare_dram_parameter("out", [S, N, C], f32, isOutput=True)

    with tile.TileContext(nc, pool_alloc_mode="queue") as tc, \
            ExitStack() as ctx:
        # ---------- pools ----------
        pers = ctx.enter_context(tc.tile_pool(name="pers", bufs=1))
        psum = ctx.enter_context(tc.tile_pool(name="psum", bufs=1, space="PSUM"))

        def ps_tile(name, wide):
            return psum.tile([128, 512], f32, name=name, tag="big", bufs=2)

        # ---------- persistent constants ----------
        ident = pers.tile([128, 128], bf16, name="ident")
        make_identity(nc, ident)
        ident8 = pers.tile([128, 128], fp8, name="ident8")
        make_identity(nc, ident8)
        bias_pack = pers.tile([128, MQK + 2 * KF + 3 * C], f32,
                              name="bias_pack")
        qkb = bias_pack[:, 0:MQK]
        b1t = bias_pack[:, MQK:MQK + KF]
        b1i = bias_pack[:, MQK + KF:MQK + 2 * KF]
        vb = bias_pack[:, MQK + 2 * KF:MQK + 2 * KF + C]
        b2t = bias_pack[:, MQK + 2 * KF + C:MQK + 2 * KF + 2 * C]
        b2i = bias_pack[:, MQK + 2 * KF + 2 * C:MQK + 2 * KF + 3 * C]
        mb = pers.tile([128, S, NT], f32, name="mb")
        eps_t = pers.tile([128, 1], f32, name="eps_t")
        nc.vector.memset(eps_t[:], EPS)

        # ---------- helpers ----------
        def layer_norm(pool, src_ap, tp, dst_ap):
            """dst(bf16) = (src - mean)/sqrt(var+EPS); src [tp, C] f32.
            bn_stats computes mean/var in one DVE pass; the normalize is a
            fused (x - mean) * rstd tensor_scalar."""
            st = pool.tile([128, 12], f32, name="ln_st", tag="ln_st", bufs=4)
            half = src_ap.rearrange("p (a b) -> p a b", b=C // 2)
            nc.vector.bn_stats(st[0:tp, 0:6], half[:, 0])
            nc.vector.bn_stats(st[0:tp, 6:12], half[:, 1])
            mv = pool.tile([128, 2], f32, name="ln_mv", tag="ln_mv", bufs=4)
            nc.vector.bn_aggr(mv[0:tp], st[0:tp])
            std = pool.tile([128, 1], f32, name="ln_std", tag="ln_std", bufs=4)
            nc.scalar.activation(std[0:tp], mv[0:tp, 1:2], Af.Sqrt,
                                 bias=eps_t[0:tp])
            rstd = pool.tile([128, 1], f32, name="ln_rstd", tag="ln_rstd", bufs=4)
            nc.vector.reciprocal(rstd[0:tp], std[0:tp])
            nc.vector.tensor_scalar(dst_ap, src_ap, mv[0:tp, 0:1], rstd[0:tp],
                                    Op.subtract, Op.mult)

        def late_dma(out_ap, in_ap):
            (nc.gpsimd if POOL_DMA else nc.sync).dma_start(out_ap, in_ap)

        tp_flip = [0]

        def transpose_block(src_full_ap, dst_full_ap, engines=None):
            """dst[128,128] = src[128,128].T via PE; evictions rotate over
            `engines` (phase-dependent: whichever engines are idle there).
            Rows beyond the valid token count carry garbage into padded dst
            columns (never read). dst dtype may differ (fp8 conversion is
            folded into the eviction)."""
            ps = psum.tile([128, 128], bf16, name="tps", tag="tp", bufs=2)
            nc.tensor.transpose(ps[:], src_full_ap, ident[:])
            engines = engines or [nc.vector, nc.scalar]
            tp_flip[0] += 1
            eng = engines[tp_flip[0] % len(engines)]
            if eng is nc.scalar:
                nc.scalar.copy(dst_full_ap, ps[:])
            else:
                eng.tensor_copy(dst_full_ap, ps[:])

        def mm_dr(ps_ap, lhs_pairs, rhs_pairs, tail=None):
            """Accumulating DoubleRow matmul over k-pairs; optional plain
            (lhsT, rhs) tail tile. lhs_pairs/rhs_pairs: list of AP pairs."""
            npair = len(lhs_pairs)
            for i, (la, ra) in enumerate(zip(lhs_pairs, rhs_pairs)):
                nc.tensor.matmul(ps_ap, la, ra, start=(i == 0),
                                 stop=(i == npair - 1 and tail is None),
                                 perf_mode=DR)
            if tail is not None:
                nc.tensor.matmul(ps_ap, tail[0], tail[1], start=False,
                                 stop=True)

        for _rep in range(REPS):
            # x2rep double-buffers across reps so rep r+1's attention era and
            # repack can overlap rep r's FFN (the marginal-rep pipeline).
            fw_ctx = tc.tile_pool(name="fw", bufs=1)
            fw = fw_ctx.__enter__()
            w1i = fw.tile([128, KC, DFF], fp8, name="w1i")
            x2rep_img = fw.tile([128, 9, C], bf16, name="x2rep_img")
            x2rep_txt = fw.tile([128, C], bf16, name="x2rep_txt")
            # ================= attention era =================
            with tc.tile_pool(name="era", bufs=1) as era:
                xT = {}
                qkT = {}
                vsb = {}
                osb = {}
                x2 = {}

                with tc.tile_pool(name="wqkv", bufs=1) as wp:
                    wqk = wp.tile([128, KC, 2 * C], fp8, name="wqk")
                    wv = wp.tile([128, KC, C], fp8, name="wv")

                    # ---- LN1 + transpose to xT ----
                    for s in range(S):
                        xT[s] = era.tile([128, KC, 640], fp8, name=f"xT{s}",
                                         tag="xT", bufs=2)
                        for (t0, tp) in TOK_TILES:
                            xin = era.tile([128, C], bf16, name="xin", tag="xin",
                                           bufs=3)
                            nc.sync.dma_start(xin[0:tp], x_d[s, t0:t0 + tp, :])
                            xh = era.tile([128, C], bf16, name="xh", tag="xh",
                                          bufs=3)
                            if tp < 128:
                                nc.vector.memset(xh[96:128, :], 0.0)
                            layer_norm(era, xin[0:tp], tp, xh[0:tp])
                            for f in range(KC):
                                transpose_block(xh[:, f * 128:(f + 1) * 128],
                                                xT[s][:, f, t0:t0 + 128],
                                                engines=[nc.scalar, nc.vector])

                    if _rep == 0:
                        nc.sync.dma_start(bias_pack[:], bp_d[:])
                        nc.sync.dma_start(mb[:],
                                          mb_d[:].rearrange("s p t -> p s t"))
                    nc.sync.dma_start(wqk[:], wqk_d[:])
                    nc.sync.dma_start(wv[:], wv_d[:])
                    # ---- QKV projections (fp8 DoubleRow over k-pairs) ----
                    for s in range(S if LVL >= 2 else 0):
                        qkT[s] = era.tile([128, MQK, N], fp8, name=f"qkT{s}",
                                          tag="qkT", bufs=2)
                        for m in range(MQK):
                            for (q0, qn) in Q_CHUNKS:
                                ps = ps_tile("ps_qk", qn)
                                mm_dr(ps[:, 0:qn],
                                      [wqk[:, 2 * k:2 * k + 2,
                                           m * 128:(m + 1) * 128]
                                       for k in range(KP)],
                                      [xT[s][:, 2 * k:2 * k + 2, q0:q0 + qn]
                                       for k in range(KP)])
                                nc.vector.tensor_scalar_add(
                                    qkT[s][:, m, q0:q0 + qn], ps[:, 0:qn],
                                    qkb[:, m:m + 1])
                        vsb[s] = era.tile([128, NT, H * HS], fp8, name=f"v{s}",
                                          tag="v", bufs=2)
                        for ti, (t0, tp) in enumerate(TOK_TILES):
                            for (n0, nn) in C_CHUNKS:
                                ps = ps_tile("ps_v", nn)
                                mm_dr(ps[0:tp, 0:nn],
                                      [xT[s][:, 2 * k:2 * k + 2, t0:t0 + tp]
                                       for k in range(KP)],
                                      [wv[:, 2 * k:2 * k + 2, n0:n0 + nn]
                                       for k in range(KP)])
                                nh = nn // 64
                                h0 = n0 // 64
                                vview = vsb[s][0:tp, ti, :].rearrange(
                                    "p (h e) -> p h e", e=HS)[:, h0:h0 + nh, 0:64]
                                nc.vector.tensor_add(
                                    vview,
                                    ps[0:tp, 0:nn].rearrange("p (h e) -> p h e",
                                                             e=64),
                                    vb[0:tp, n0:n0 + nn].rearrange(
                                        "p (h e) -> p h e", e=64))
                            ones = vsb[s][0:tp, ti, :].rearrange(
                                "p (h e) -> p h e", e=HS)[:, :, 64:65]
                            nc.vector.memset(ones, 1.0)

                # ---- prefetch FFN weights on the SWDGE queue (DMA-idle
                # stretch of the attention core) ----
                if LVL >= 5:
                    for k in range(KC):
                        late_dma(w1i[:, k, :], w1i_d[:, k, :])

                # ---- attention core + proj, s-outer: sample 0's proj /
                # repack (PE/DVE/DMA) overlaps sample 1's exp-bound
                # attention. rpb is re-loaded per sample (fp8, cheap).
                for s in range(S if LVL >= 3 else 0):
                    osb[s] = era.tile([128, NT, C], bf16, name=f"o{s}",
                                      tag="o", bufs=2)
                    nc.vector.memset(osb[s][96:128, NT - 1, :], 0.0)
                with tc.tile_pool(name="attn", bufs=1) as apool, \
                        tc.tile_pool(name="proj", bufs=1) as pp:
                    wpj = pp.tile([128, KC, C], fp8, name="wpj")
                    nc.sync.dma_start(wpj[:], wpj_d[:])
                    for s in range(S if LVL >= 3 else 0):
                        for h in range(H):
                            rpb = apool.tile([128, NT, N], fp8, name="rpb",
                                             tag="rpb", bufs=2)
                            nc.sync.dma_start(rpb[:], rpb_d[h])
                            mtile = KC + h // 2
                            qtile = h // 2
                            base = (h % 2) * 64
                            expT = apool.tile([128, NT, NPAD], fp8, name="expT",
                                              tag="expT", bufs=2)
                            for kt, (k0, tp) in enumerate(TOK_TILES):
                                # [128, 640] f32 = 1.25 PSUM banks; each
                                # matmul chunk stays within one bank (512
                                # boundary), the Exp eviction reads across.
                                ps = psum.tile([128, NPAD], f32, name="ps_sc",
                                               tag="sc", bufs=2)
                                for (q0, qn) in Q_CHUNKS:
                                    nc.tensor.matmul(
                                        ps[0:tp, q0:q0 + qn],
                                        qkT[s][base:base + 64, mtile, k0:k0 + tp],
                                        qkT[s][base:base + 64, qtile, q0:q0 + qn],
                                        start=True, stop=False)
                                    nc.tensor.matmul(
                                        ps[0:tp, q0:q0 + qn],
                                        ident8[0:tp, 0:tp],
                                        rpb[0:tp, kt, q0:q0 + qn],
                                        start=False, stop=True)
                                nc.scalar.activation(
                                    expT[0:tp, kt, 0:N],
                                    ps[0:tp, 0:N], Af.Exp,
                                    bias=mb[0:tp, s, kt:kt + 1])
                            for qt, (qq0, qp) in enumerate(TOK_TILES):
                                ops = psum.tile([128, 65], f32, name="ops",
                                                tag="tp", bufs=2)
                                mm_dr(ops[0:qp, :],
                                      [expT[:, 2 * j:2 * j + 2, qq0:qq0 + qp]
                                       for j in range(2)],
                                      [vsb[s][:, 2 * j:2 * j + 2,
                                              h * HS:h * HS + 65]
                                       for j in range(2)],
                                      tail=(expT[0:104, 4, qq0:qq0 + qp],
                                            vsb[s][0:104, 4, h * HS:h * HS + 65]))
                                rc = era.tile([128, 1], f32, name="rc", tag="rc",
                                              bufs=4)
                                nc.vector.reciprocal(rc[0:qp], ops[0:qp, 64:65])
                                nc.vector.tensor_scalar_mul(
                                    osb[s][0:qp, qt, h * 64:(h + 1) * 64],
                                    ops[0:qp, 0:64], rc[0:qp])

                        # ---- proj + residual for sample s (fp8 DoubleRow) ----
                        if LVL < 4:
                            continue
                        oT = era.tile([128, KC, 640], fp8, name=f"oT{s}",
                                      tag="xT", bufs=2)
                        for ti, (t0, tp) in enumerate(TOK_TILES):
                            for f in range(KC):
                                transpose_block(
                                    osb[s][:, ti, f * 128:(f + 1) * 128],
                                    oT[:, f, t0:t0 + 128],
                                    engines=[nc.scalar, nc.vector])
                        x2[s] = era.tile([128, NT, C], bf16, name=f"x2_{s}",
                                         tag="x2", bufs=2)
                        for ti, (t0, tp) in enumerate(TOK_TILES):
                            xres = pp.tile([128, C], bf16, name="xres", tag="xres",
                                           bufs=2)
                            late_dma(xres[0:tp], xb_d[s, t0:t0 + tp, :])
                            for (n0, nn) in C_CHUNKS:
                                ps = ps_tile("ps_pj", nn)
                                mm_dr(ps[0:tp, 0:nn],
                                      [oT[:, 2 * k:2 * k + 2, t0:t0 + tp]
                                       for k in range(KP)],
                                      [wpj[:, 2 * k:2 * k + 2, n0:n0 + nn]
                                       for k in range(KP)])
                                nc.vector.tensor_add(
                                    x2[s][0:tp, ti, n0:n0 + nn],
                                    ps[0:tp, 0:nn], xres[0:tp, n0:n0 + nn])
                        # ---- repack x2[s] -> text [80, C] + img rows ----
                        nc.sync.dma_start(x2rep_txt[40 * s:40 * s + 40, :],
                                          x2[s][0:40, 0, :])
                        g = 576 * s
                        for kt, (t0, tp) in enumerate(TOK_TILES):
                            p0 = 40 if kt == 0 else 0
                            length = tp - p0
                            src_off = p0
                            while length > 0:
                                j, dp = g // 128, g % 128
                                piece = min(length, 128 - dp)
                                nc.sync.dma_start(
                                    x2rep_img[dp:dp + piece, j, :],
                                    x2[s][src_off:src_off + piece, kt, :])
                                g += piece
                                src_off += piece
                                length -= piece

            # ================= FFN era =================
            if LVL >= 5:
                with tc.tile_pool(name="ffn", bufs=1) as fp:
                    w2i = fp.tile([128, KF, C], fp8, name="w2i")
                    nc.sync.dma_start(w2i[:, 0:12, :], w2i_d[:, 0:12, :])
                    nc.sync.dma_start(w2i[:, 12:24, :], w2i_d[:, 12:24, :])
                    # LN2 + transpose
                    ztT = fp.tile([128, KC, 128], fp8, name="ztT")
                    xh2 = fp.tile([128, C], bf16, name="xh2", tag="xh2", bufs=2)
                    nc.vector.memset(xh2[64:128, :], 0.0)
                    layer_norm(fp, x2rep_txt[0:TXTTOK], TXTTOK, xh2[0:TXTTOK])
                    for f in range(KC):
                        transpose_block(xh2[:, f * 128:(f + 1) * 128],
                                        ztT[:, f, 0:128])
                    ziT = fp.tile([128, KC, IMGTOK], fp8, name="ziT")
                    for j in range(9):
                        xh2 = fp.tile([128, C], bf16, name="xh2", tag="xh2", bufs=2)
                        layer_norm(fp, x2rep_img[:, j, :], 128, xh2[:])
                        for f in range(KC):
                            transpose_block(xh2[:, f * 128:(f + 1) * 128],
                                            ziT[:, f, j * 128:(j + 1) * 128],
                                            engines=[nc.vector, nc.vector,
                                                     nc.scalar])
                    # Pre-add the (gamma_2-folded) fc2 biases into the residual so
                    # each fc2 eviction is a single tensor_add. In-place; Tile
                    # orders these after the LN2 reads above.
                    nc.gpsimd.tensor_add(x2rep_txt[0:TXTTOK, :], x2rep_txt[0:TXTTOK, :],
                                         b2t[0:TXTTOK, :])
                    for j in range(9):
                        nc.gpsimd.tensor_add(x2rep_img[:, j, :], x2rep_img[:, j, :],
                                             b2i[:, :])

                    # ---- img FFN (resident weights, 3 token chunks) ----
                    for c in range(3):
                        q0 = c * IMG_CHUNK
                        hgi = fp.tile([128, KF, IMG_CHUNK], fp8, name="hgi",
                                      tag="hgi", bufs=2)
                        for m in range(KF):
                            ps = ps_tile("ps_f1i", 512)
                            mm_dr(ps[:, 0:IMG_CHUNK],
                                  [w1i[:, 2 * k:2 * k + 2,
                                       m * 128:(m + 1) * 128]
                                   for k in range(KP)],
                                  [ziT[:, 2 * k:2 * k + 2, q0:q0 + IMG_CHUNK]
                                   for k in range(KP)])
                            nc.scalar.activation(hgi[:, m, :], ps[:, 0:IMG_CHUNK],
                                                 Af.Gelu, bias=b1i[:, m:m + 1])
                        for mt in range(3):
                            j = 3 * c + mt
                            # fc2 accumulators live on the "sc" banks (idle
                            # outside attention) so fc1's "big" ring flows.
                            ps0 = psum.tile([128, NPAD], f32, name="ps_f2i0",
                                            tag="sc", bufs=2)
                            ps1 = psum.tile([128, NPAD], f32, name="ps_f2i1",
                                            tag="sc", bufs=2)
                            mm_dr(ps0[:, 0:512],
                                  [hgi[:, 2 * k:2 * k + 2,
                                       mt * 128:(mt + 1) * 128]
                                   for k in range(KFP)],
                                  [w2i[:, 2 * k:2 * k + 2, 0:512]
                                   for k in range(KFP)])
                            mm_dr(ps1[:, 0:256],
                                  [hgi[:, 2 * k:2 * k + 2,
                                       mt * 128:(mt + 1) * 128]
                                   for k in range(KFP)],
                                  [w2i[:, 2 * k:2 * k + 2, 512:768]
                                   for k in range(KFP)])
                            ot = fp.tile([128, C], f32, name="ot", tag="ost", bufs=3)
                            for (n0, nn), ps in zip(C_CHUNKS, [ps0, ps1]):
                                nc.vector.tensor_add(ot[:, n0:n0 + nn], ps[:, 0:nn],
                                                     x2rep_img[:, j, n0:n0 + nn])
                            # DMA out: global img row g = 128*j -> (b, 40 + g%576)
                            g0 = 128 * j
                            p = 0
                            while p < 128:
                                g = g0 + p
                                b = g // IMG
                                piece = min(128 - p, IMG * (b + 1) - g)
                                late_dma(
                                    out_d[b, TXT + g - b * IMG:
                                          TXT + g - b * IMG + piece, :],
                                    ot[p:p + piece, :])
                                p += piece

                    # ---- text FFN (streamed weights, fp8 DoubleRow) ----
                    with tc.tile_pool(name="wtxt", bufs=1) as wt:
                        hgt = fp.tile([128, KF, TXTTOK], fp8, name="hgt")
                        for mc in range(8):
                            w1tc = wt.tile([128, 3, KC, 128], fp8,
                                           name="w1tc", tag="w1tc", bufs=2)
                            nc.sync.dma_start(
                                w1tc[:],
                                w1t_d[3 * mc:3 * mc + 3].rearrange(
                                    "m p k n -> p m k n"))
                            for ml in range(3):
                                m = 3 * mc + ml
                                ps = ps_tile("ps_f1t", 512)
                                mm_dr(ps[:, 0:TXTTOK],
                                      [w1tc[:, ml, 2 * k:2 * k + 2, :]
                                       for k in range(KP)],
                                      [ztT[:, 2 * k:2 * k + 2, 0:TXTTOK]
                                       for k in range(KP)])
                                nc.scalar.activation(
                                    hgt[:, m, 0:TXTTOK], ps[:, 0:TXTTOK],
                                    Af.Gelu, bias=b1t[:, m:m + 1])
                        ps0 = psum.tile([128, NPAD], f32, name="ps_f2t0",
                                        tag="sc", bufs=2)
                        ps1 = psum.tile([128, NPAD], f32, name="ps_f2t1",
                                        tag="sc", bufs=2)
                        for kc4 in range(6):
                            w2tc = wt.tile([128, 4, C], fp8, name="w2tc",
                                           tag="w2tc", bufs=2)
                            nc.sync.dma_start(
                                w2tc[:],
                                w2t_d[4 * kc4:4 * kc4 + 4].rearrange(
                                    "k p n -> p k n"))
                            for kl in range(2):
                                kp = 2 * kc4 + kl     # global k-pair index
                                nc.tensor.matmul(
                                    ps0[0:TXTTOK, 0:512],
                                    hgt[:, 2 * kp:2 * kp + 2, 0:TXTTOK],
                                    w2tc[:, 2 * kl:2 * kl + 2, 0:512],
                                    start=(kp == 0), stop=(kp == KFP - 1),
                                    perf_mode=DR)
                                nc.tensor.matmul(
                                    ps1[0:TXTTOK, 0:256],
                                    hgt[:, 2 * kp:2 * kp + 2, 0:TXTTOK],
                                    w2tc[:, 2 * kl:2 * kl + 2, 512:768],
                                    start=(kp == 0), stop=(kp == KFP - 1),
                                    perf_mode=DR)
                        ot = fp.tile([128, C], f32, name="ot", tag="ost", bufs=3)
                        for (n0, nn), ps in zip(C_CHUNKS, [ps0, ps1]):
                            nc.vector.tensor_add(ot[0:TXTTOK, n0:n0 + nn],
                                                 ps[0:TXTTOK, 0:nn],
                                                 x2rep_txt[0:TXTTOK, n0:n0 + nn])
                        for s in range(S):
                            late_dma(out_d[s, 0:TXT, :],
                                     ot[40 * s:40 * s + 40, :])
            fw_ctx.__exit__(None, None, None)

    nc.compile()
    return nc


_CACHE = {}


def _get_program():
    if "nc" not in _CACHE:
        _CACHE["nc"] = build_program()
    return _CACHE["nc"]


def run(inputs, trace=False):
    from concourse.bass_utils import run_bass_kernel_spmd
    shared, per_core = host_prep(inputs)
    nc = _get_program()
    in_maps = [{**shared, **pc} for pc in per_core]
    res = run_bass_kernel_spmd(nc, in_maps, core_ids=list(range(NCORES)),
                               trace=trace)
    out = np.concatenate([res.results[i]["out"] for i in range(NCORES)],
                         axis=0).astype(np.float32)
    return out, res


def kernel(**inputs):
    out, _ = run(inputs, trace=False)
    return out


# revision 42
# speedup vs baseline: 2.3912x; 1.7995x over previous
"""Trainium2 Bass kernel for nn_Block_17540646437178 (dense transformer block).

Sharding: data-parallel over B=16 across 8 NeuronCores (2 samples/core,
zero collectives).

v2: all large matmuls (QKV, V, proj, both FFNs, attn@V) run in fp8e4m3
with DoubleRow perf mode (256-deep contraction pairs, f32 PSUM accum);
scores (K=64) stay bf16. PE transposes stay bf16 (fp8 transpose is
broken in the toolchain); fp8 conversion is folded into the PSUM
evictions that happen anyway. Attention loops h-outer/s-inner so each
exp(rpb) slab is DMA'd once and reused for both samples.

Host-side folding (exact, f32): layernorm affines fold into the following
matmul weights/biases; the attention scale folds into W_q; gamma_1/gamma_2
fold into w_proj/fc2; the proj bias folds into a pre-biased residual copy
of x ("xb"); all remaining small biases ship as one packed [128, X] tile.

Attention layout: scores are computed TRANSPOSED (k-tokens on partitions)
so (a) the key-padding mask becomes a per-partition Exp bias, (b) softmax
needs no max-subtraction (logits are O(1); masked lanes underflow to 0),
(c) exp(s + rpb + mask) = exp(s + mask) * exp(rpb) with exp(rpb)
precomputed on host (bf16 multiply on DVE, fp8 output). V carries an
appended ones-column so the softmax denominator falls out of the attn@V
matmul, landing per-partition for the normalize multiply.

The text/img FFN split (tokens 0:40 vs 40:616) is handled by DMA-repacking
the post-attention residual into [80, C] and [1152 = 9x128, C] buffers so
every FFN matmul is 128-aligned.
"""

import numpy as np
import ml_dtypes

BF16NP = ml_dtypes.bfloat16
F8NP = ml_dtypes.float8_e4m3

B, N, C, H, D = 16, 616, 768, 12, 64
TXT = 40
DFF = 3072
NCORES = 8
S = B // NCORES          # 2 samples per core
EPS = 1e-5
SCALE = D ** -0.5
KC = C // 128            # 6 k-tiles over C
KP = KC // 2             # 3 DoubleRow k-pairs over C
MQK = (2 * C) // 128     # 12 m-tiles over q+k features
KF = DFF // 128          # 24 k-tiles over dff
KFP = KF // 2            # 12 DoubleRow k-pairs over dff
NT = 5                   # token tiles per sample (616 = 4*128 + 104)
TOK_TILES = [(0, 128), (128, 128), (256, 128), (384, 128), (512, 104)]
Q_CHUNKS = [(0, 512), (512, 104)]    # 616 free-dim chunks
C_CHUNKS = [(0, 512), (512, 256)]    # 768 free-dim chunks
IMG = N - TXT            # 576
IMGTOK = S * IMG         # 1152 = 9*128
TXTTOK = S * TXT         # 80
IMG_CHUNK = 384          # img token chunk for FFN (3 chunks)
NEG = -30000.0
HS = 80                  # vsb per-head stride (64 V + 1 ones + pad, %16==0)
NPAD = 640               # expT row stride (%16==0)


def _slab_kp(wt, np_dt):
    """[K, M] (K = KT*128) -> [128, KT, M] slab layout (partition-major)."""
    k, m = wt.shape
    kt = k // 128
    assert kt * 128 == k
    return np.ascontiguousarray(
        wt.astype(np.float32).reshape(kt, 128, m).transpose(1, 0, 2)).astype(np_dt)


def _bf(a):
    return np.ascontiguousarray(np.asarray(a, np.float32)).astype(BF16NP)


def _f32(a):
    return np.ascontiguousarray(np.asarray(a, dtype=np.float32))


def _bcast128(v):
    return np.ascontiguousarray(np.broadcast_to(v.astype(np.float32), (128, v.shape[0])))


def _colmajor(v, nt):
    """(nt*128,) -> [128, nt] with column t holding partitions of tile t."""
    return np.ascontiguousarray(v.astype(np.float32).reshape(nt, 128).T)


def host_prep(inputs):
    """Fold affines/scales into weights; build slab/broadcast layouts.

    Returns (shared, per_core) where per_core is a list of dicts.
    """
    inp = {k: _f32(v) if np.asarray(v).dtype != np.int32 else np.asarray(v)
           for k, v in inputs.items()}

    g1, g2 = inp["gamma_1"], inp["gamma_2"]

    # --- attention: fold ln1 affine + SCALE into w_qkv ---
    wqkv = inp["w_qkv"] * inp["ln1_g"][None, :]
    qkv_b = np.concatenate([inp["q_bias"],
                            np.zeros_like(inp["v_bias"]),
                            inp["v_bias"]])
    qkv_b = qkv_b + inp["w_qkv"] @ inp["ln1_b"]
    wqkv[:C] *= SCALE
    qkv_b[:C] *= SCALE

    # Weights are host-scaled by powers of 2 out of fp8's subnormal range
    # (std 0.02 ~ 2^-5.6); the inverse scale folds into each eviction.
    w_qk = _slab_kp(16.0 * wqkv[: 2 * C].T, F8NP)    # [128, 6, 1536] fp8
    w_v = _slab_kp(16.0 * wqkv[2 * C:].T, F8NP)      # [128, 6, 768] fp8
    qk_bias = _colmajor(qkv_b[: 2 * C], MQK)         # [128, 12] f32
    v_bias = _bcast128(16.0 * qkv_b[2 * C:])         # [128, 768] f32 (x16)

    # --- proj: fold gamma_1 ---
    wpj = g1[:, None] * inp["w_proj"]
    w_pj = _slab_kp(8.0 * wpj.T, F8NP)               # [128, 6, 768] fp8 (x8)
    b_pj = _bcast128(g1 * inp["b_proj"])             # [128, 768] f32

    # --- FFN branches: fold ln2 affine into fc1, gamma_2 into fc2 ---
    def ffn(w1, b1, w2, b2, lg, lb):
        w1e = w1 * lg[None, :]
        b1e = b1 + w1 @ lb
        w2e = g2[:, None] * w2
        b2e = g2 * b2
        return w1e, b1e, w2e, b2e

    w1t, b1t, w2t, b2t = ffn(inp["fc1t_w"], inp["fc1t_b"], inp["fc2t_w"],
                             inp["fc2t_b"], inp["ln2t_g"], inp["ln2t_b"])
    w1i, b1i, w2i, b2i = ffn(inp["fc1i_w"], inp["fc1i_b"], inp["fc2i_w"],
                             inp["fc2i_b"], inp["ln2i_g"], inp["ln2i_b"])

    # text fc1 weights grouped by M-slab for streaming: [24, 128, 6, 128] fp8
    w1t_T = 16.0 * w1t.T.astype(np.float32)          # [768, 3072] (x16)
    w1t_m = np.ascontiguousarray(
        w1t_T.reshape(KC, 128, KF, 128).transpose(2, 1, 0, 3)).astype(F8NP)
    w2t_k = np.ascontiguousarray(
        w2t.T.astype(np.float32).reshape(KF, 128, C)).astype(F8NP)  # [24,128,768]
    w1i_s = _slab_kp(16.0 * w1i.T, F8NP)             # [128, 6, 3072] (x16)
    w2i_s = _slab_kp(w2i.T, F8NP)                    # [128, 24, 768]
    b1t_c = _colmajor(b1t, KF)                       # [128, 24]
    b1i_c = _colmajor(b1i, KF)
    b2t_b = _bcast128(b2t)                           # [128, 768]
    b2i_b = _bcast128(b2i)

    # --- rpb transposed + k-padded slabs: [12, 128, 5, 616] bf16.
    # rpb is accumulated into the scores PSUM via an identity matmul, so
    # the ACT Exp eviction computes exp(s + rpb + maskb) directly (fp8 out).
    rpbT = np.transpose(inp["relative_position_bias"], (0, 2, 1))  # [H, k, q]
    rpb_pad = np.zeros((H, NT * 128, N), np.float32)
    rpb_pad[:, :N, :] = rpbT
    rpb_slab = np.ascontiguousarray(
        rpb_pad.reshape(H, NT, 128, N).transpose(0, 2, 1, 3)).astype(F8NP)

    bias_pack = np.concatenate(
        [qk_bias, b1t_c, b1i_c, v_bias, b2t_b, b2i_b], axis=1)
    shared = dict(w_qk=w_qk, w_v=w_v, bias_pack=np.ascontiguousarray(bias_pack),
                  w_pj=w_pj, rpb=rpb_slab, w1t=w1t_m, w2t=w2t_k,
                  w1i=w1i_s, w2i=w2i_s)

    # --- per-core: x shard + mask bias ---
    mask = np.asarray(inputs["mask"]).astype(np.float32)   # [B, N] 0/1
    mb_full = (1.0 - mask) * NEG                            # [B, N]
    mb_pad = np.full((B, NT * 128), NEG, np.float32)
    mb_pad[:, :N] = mb_full
    # xb = x with the (gamma_1-folded) proj bias pre-added: the proj
    # residual eviction then needs a single tensor_add.
    xb_full = inp["x"] + (g1 * inp["b_proj"])[None, None, :]
    per_core = []
    for c in range(NCORES):
        xs = np.ascontiguousarray(inp["x"][c * S:(c + 1) * S]).astype(BF16NP)
        xbs = np.ascontiguousarray(xb_full[c * S:(c + 1) * S]).astype(BF16NP)
        mb = np.ascontiguousarray(
            mb_pad[c * S:(c + 1) * S].reshape(S, NT, 128).transpose(0, 2, 1))
        per_core.append(dict(x=xs, xb=xbs, maskb=mb))
    return shared, per_core


def build_program(ablate=None):
    """Build the per-core Bass/Tile program. Returns compiled nc.

    ablate: None/"full", or one of "ln","qkv","attn","proj" to stop
    emission after that phase (timing experiments only — output garbage).
    """
    import os
    if ablate is None:
        ablate = os.environ.get("KERNEL_ABLATE", "full")
    LVL = {"ln": 1, "qkv": 2, "attn": 3, "proj": 4, "full": 9}[ablate]
    off = set(os.environ.get("KERNEL_OFF", "").split(","))
    POOL_DMA = "pooldma" not in off   # late DMAs on GpSimd SWDGE
    REPS = int(os.environ.get("KERNEL_REPS", "1"))
    from contextlib import ExitStack
    import concourse.bass as bass
    import concourse.mybir as mybir
    import concourse.tile as tile
    from concourse import bacc
    from concourse.masks import make_identity

    f32 = mybir.dt.float32
    bf16 = mybir.dt.bfloat16
    fp8 = mybir.dt.float8e4
    DR = mybir.MatmulPerfMode.DoubleRow
    Af = mybir.ActivationFunctionType
    Ax = mybir.AxisListType
    Op = mybir.AluOpType

    nc = bacc.Bacc("TRN2", target_bir_lowering=False, debug=False,
                   num_devices=NCORES)

    x_d = nc.declare_dram_parameter("x", [S, N, C], bf16, isOutput=False)
    xb_d = nc.declare_dram_parameter("xb", [S, N, C], bf16, isOutput=False)
    mb_d = nc.declare_dram_parameter("maskb", [S, 128, NT], f32, isOutput=False)
    wqk_d = nc.declare_dram_parameter("w_qk", [128, KC, 2 * C], fp8, isOutput=False)
    wv_d = nc.declare_dram_parameter("w_v", [128, KC, C], fp8, isOutput=False)
    bp_d = nc.declare_dram_parameter("bias_pack",
                                     [128, MQK + 2 * KF + 3 * C], f32,
                                     isOutput=False)
    wpj_d = nc.declare_dram_parameter("w_pj", [128, KC, C], fp8, isOutput=False)
    rpb_d = nc.declare_dram_parameter("rpb", [H, 128, NT, N], fp8, isOutput=False)
    w1t_d = nc.declare_dram_parameter("w1t", [KF, 128, KC, 128], fp8, isOutput=False)
    w2t_d = nc.declare_dram_parameter("w2t", [KF, 128, C], fp8, isOutput=False)
    w1i_d = nc.declare_dram_parameter("w1i", [128, KC, DFF], fp8, isOutput=False)
    w2i_d = nc.declare_dram_parameter("w2i", [128, KF, C], fp8, isOutput=False)
    out_d = nc.declare_dram_parameter("out", [S, N, C], f32, isOutput=True)

    with tile.TileContext(nc, pool_alloc_mode="queue") as tc, \
            ExitStack() as ctx:
        # ---------- pools ----------
        pers = ctx.enter_context(tc.tile_pool(name="pers", bufs=1))
        psum = ctx.enter_context(tc.tile_pool(name="psum", bufs=1, space="PSUM"))

        def ps_tile(name, wide):
            return psum.tile([128, 512], f32, name=name, tag="big", bufs=2)

        # ---------- persistent constants ----------
        ident = pers.tile([128, 128], bf16, name="ident")
        make_identity(nc, ident)
        ident8 = pers.tile([128, 128], fp8, name="ident8")
        make_identity(nc, ident8)
        bias_pack = pers.tile([128, MQK + 2 * KF + 3 * C], f32,
                              name="bias_pack")
        qkb = bias_pack[:, 0:MQK]
        b1t = bias_pack[:, MQK:MQK + KF]
        b1i = bias_pack[:, MQK + KF:MQK + 2 * KF]
        vb = bias_pack[:, MQK + 2 * KF:MQK + 2 * KF + C]
        b2t = bias_pack[:, MQK + 2 * KF + C:MQK + 2 * KF + 2 * C]
        b2i = bias_pack[:, MQK + 2 * KF + 2 * C:MQK + 2 * KF + 3 * C]
        mb = pers.tile([128, S, NT], f32, name="mb")
        eps_t = pers.tile([128, 1], f32, name="eps_t")
        nc.vector.memset(eps_t[:], EPS)

        # ---------- helpers ----------
        def layer_norm(pool, src_ap, tp, dst_ap):
            """dst(bf16) = (src - mean)/sqrt(var+EPS); src [tp, C] f32.
            bn_stats computes mean/var in one DVE pass; the normalize is a
            fused (x - mean) * rstd tensor_scalar."""
            st = pool.tile([128, 12], f32, name="ln_st", tag="ln_st", bufs=4)
            half = src_ap.rearrange("p (a b) -> p a b", b=C // 2)
            nc.vector.bn_stats(st[0:tp, 0:6], half[:, 0])
            nc.vector.bn_stats(st[0:tp, 6:12], half[:, 1])
            mv = pool.tile([128, 2], f32, name="ln_mv", tag="ln_mv", bufs=4)
            nc.vector.bn_aggr(mv[0:tp], st[0:tp])
            std = pool.tile([128, 1], f32, name="ln_std", tag="ln_std", bufs=4)
            nc.scalar.activation(std[0:tp], mv[0:tp, 1:2], Af.Sqrt,
                                 bias=eps_t[0:tp])
            rstd = pool.tile([128, 1], f32, name="ln_rstd", tag="ln_rstd", bufs=4)
            nc.vector.reciprocal(rstd[0:tp], std[0:tp])
            nc.vector.tensor_scalar(dst_ap, src_ap, mv[0:tp, 0:1], rstd[0:tp],
                                    Op.subtract, Op.mult)

        def late_dma(out_ap, in_ap):
            (nc.gpsimd if POOL_DMA else nc.sync).dma_start(out_ap, in_ap)

        tp_flip = [0]

        def transpose_block(src_full_ap, dst_full_ap, engines=None, scale=None):
            """dst[128,128] = src[128,128].T via PE; evictions rotate over
            `engines` (phase-dependent: whichever engines are idle there).
            Rows beyond the valid token count carry garbage into padded dst
            columns (never read). dst dtype may differ (fp8 conversion is
            folded into the eviction)."""
            ps = psum.tile([128, 128], bf16, name="tps", tag="tp", bufs=2)
            nc.tensor.transpose(ps[:], src_full_ap, ident[:])
            engines = engines or [nc.vector, nc.scalar]
            tp_flip[0] += 1
            eng = engines[tp_flip[0] % len(engines)]
            if eng is nc.scalar:
                if scale is None:
                    nc.scalar.copy(dst_full_ap, ps[:])
                else:
                    nc.scalar.mul(dst_full_ap, ps[:], scale)
            else:
                if scale is None:
                    eng.tensor_copy(dst_full_ap, ps[:])
                else:
                    eng.tensor_scalar_mul(dst_full_ap, ps[:], scale)

        def mm_dr(ps_ap, lhs_pairs, rhs_pairs, tail=None):
            """Accumulating DoubleRow matmul over k-pairs; optional plain
            (lhsT, rhs) tail tile. lhs_pairs/rhs_pairs: list of AP pairs."""
            npair = len(lhs_pairs)
            for i, (la, ra) in enumerate(zip(lhs_pairs, rhs_pairs)):
                nc.tensor.matmul(ps_ap, la, ra, start=(i == 0),
                                 stop=(i == npair - 1 and tail is None),
                                 perf_mode=DR)
            if tail is not None:
                nc.tensor.matmul(ps_ap, tail[0], tail[1], start=False,
                                 stop=True)

        for _rep in range(REPS):
            # x2rep double-buffers across reps so rep r+1's attention era and
            # repack can overlap rep r's FFN (the marginal-rep pipeline).
            fw_ctx = tc.tile_pool(name="fw", bufs=1)
            fw = fw_ctx.__enter__()
            w1i = fw.tile([128, KC, DFF], fp8, name="w1i")
            x2rep_img = fw.tile([128, 9, C], bf16, name="x2rep_img")
            x2rep_txt = fw.tile([128, C], bf16, name="x2rep_txt")
            # ================= attention era =================
            with tc.tile_pool(name="era", bufs=1) as era:
                xT = {}
                qkT = {}
                vsb = {}
                osb = {}
                x2 = {}

                with tc.tile_pool(name="wqkv", bufs=1) as wp:
                    wqk = wp.tile([128, KC, 2 * C], fp8, name="wqk")
                    wv = wp.tile([128, KC, C], fp8, name="wv")

                    # ---- LN1 + transpose to xT ----
                    for s in range(S):
                        xT[s] = era.tile([128, KC, 640], fp8, name=f"xT{s}",
                                         tag="xT", bufs=2)
                        for (t0, tp) in TOK_TILES:
                            xin = era.tile([128, C], bf16, name="xin", tag="xin",
                                           bufs=3)
                            nc.sync.dma_start(xin[0:tp], x_d[s, t0:t0 + tp, :])
                            xh = era.tile([128, C], bf16, name="xh", tag="xh",
                                          bufs=3)
                            if tp < 128:
                                nc.vector.memset(xh[96:128, :], 0.0)
                            layer_norm(era, xin[0:tp], tp, xh[0:tp])
                            for f in range(KC):
                                transpose_block(xh[:, f * 128:(f + 1) * 128],
                                                xT[s][:, f, t0:t0 + 128],
                                                engines=[nc.scalar, nc.scalar,
                                                         nc.vector])

                    if _rep == 0:
                        nc.sync.dma_start(bias_pack[:], bp_d[:])
                        nc.sync.dma_start(mb[:],
                                          mb_d[:].rearrange("s p t -> p s t"))
                    nc.sync.dma_start(wqk[:], wqk_d[:])
                    nc.sync.dma_start(wv[:], wv_d[:])
                    # ---- QKV projections (fp8 DoubleRow over k-pairs) ----
                    for s in range(S if LVL >= 2 else 0):
                        qkT[s] = era.tile([128, MQK, N], fp8, name=f"qkT{s}",
                                          tag="qkT", bufs=2)
                        for m in range(MQK):
                            for (q0, qn) in Q_CHUNKS:
                                ps = ps_tile("ps_qk", qn)
                                mm_dr(ps[:, 0:qn],
                                      [wqk[:, 2 * k:2 * k + 2,
                                           m * 128:(m + 1) * 128]
                                       for k in range(KP)],
                                      [xT[s][:, 2 * k:2 * k + 2, q0:q0 + qn]
                                       for k in range(KP)])
                                nc.vector.tensor_scalar(
                                    qkT[s][:, m, q0:q0 + qn], ps[:, 0:qn],
                                    1.0 / 16.0, qkb[:, m:m + 1],
                                    Op.mult, Op.add)
                        vsb[s] = era.tile([128, NT, H * HS], fp8, name=f"v{s}",
                                          tag="v", bufs=2)
                        for ti, (t0, tp) in enumerate(TOK_TILES):
                            for (n0, nn) in C_CHUNKS:
                                ps = ps_tile("ps_v", nn)
                                mm_dr(ps[0:tp, 0:nn],
                                      [xT[s][:, 2 * k:2 * k + 2, t0:t0 + tp]
                                       for k in range(KP)],
                                      [wv[:, 2 * k:2 * k + 2, n0:n0 + nn]
                                       for k in range(KP)])
                                nh = nn // 64
                                h0 = n0 // 64
                                vview = vsb[s][0:tp, ti, :].rearrange(
                                    "p (h e) -> p h e", e=HS)[:, h0:h0 + nh, 0:64]
                                nc.vector.tensor_add(
                                    vview,
                                    ps[0:tp, 0:nn].rearrange("p (h e) -> p h e",
                                                             e=64),
                                    vb[0:tp, n0:n0 + nn].rearrange(
                                        "p (h e) -> p h e", e=64))
                            ones = vsb[s][0:tp, ti, :].rearrange(
                                "p (h e) -> p h e", e=HS)[:, :, 64:65]
                            nc.vector.memset(ones, 1.0)

                # ---- prefetch FFN weights on the SWDGE queue (DMA-idle
                # stretch of the attention core) ----
                if LVL >= 5:
                    for k in range(KC):
                        late_dma(w1i[:, k, :], w1i_d[:, k, :])

                # ---- attention core + proj, s-outer: sample 0's proj /
                # repack (PE/DVE/DMA) overlaps sample 1's exp-bound
                # attention. rpb is re-loaded per sample (fp8, cheap).
                for s in range(S if LVL >= 3 else 0):
                    osb[s] = era.tile([128, NT, C], bf16, name=f"o{s}",
                                      tag="o", bufs=2)
                    nc.vector.memset(osb[s][96:128, NT - 1, :], 0.0)
                with tc.tile_pool(name="attn", bufs=1) as apool, \
                        tc.tile_pool(name="proj", bufs=1) as pp:
                    wpj = pp.tile([128, KC, C], fp8, name="wpj")
                    nc.sync.dma_start(wpj[:], wpj_d[:])
                    for s in range(S if LVL >= 3 else 0):
                        for h in range(H):
                            if h % 2 == 0:
                                rpb2 = apool.tile([128, 2, NT, N], fp8,
                                                  name="rpb2", tag="rpb",
                                                  bufs=2)
                                nc.sync.dma_start(
                                    rpb2[:],
                                    rpb_d[h:h + 2].rearrange(
                                        "h p t n -> p h t n"))
                            rpb = rpb2[:, h % 2]
                            mtile = KC + h // 2
                            qtile = h // 2
                            base = (h % 2) * 64
                            expT = apool.tile([128, NT, NPAD], fp8, name="expT",
                                              tag="expT", bufs=2)
                            for kt, (k0, tp) in enumerate(TOK_TILES):
                                # [128, 640] f32 = 1.25 PSUM banks; each
                                # matmul chunk stays within one bank (512
                                # boundary), the Exp eviction reads across.
                                ps = psum.tile([128, NPAD], f32, name="ps_sc",
                                               tag="sc", bufs=2)
                                for (q0, qn) in Q_CHUNKS:
                                    nc.tensor.matmul(
                                        ps[0:tp, q0:q0 + qn],
                                        qkT[s][base:base + 64, mtile, k0:k0 + tp],
                                        qkT[s][base:base + 64, qtile, q0:q0 + qn],
                                        start=True, stop=False)
                                    nc.tensor.matmul(
                                        ps[0:tp, q0:q0 + qn],
                                        ident8[0:tp, 0:tp],
                                        rpb[0:tp, kt, q0:q0 + qn],
                                        start=False, stop=True)
                                nc.scalar.activation(
                                    expT[0:tp, kt, 0:N],
                                    ps[0:tp, 0:N], Af.Exp,
                                    bias=mb[0:tp, s, kt:kt + 1])
                            for qt, (qq0, qp) in enumerate(TOK_TILES):
                                ops = psum.tile([128, 65], f32, name="ops",
                                                tag="tp", bufs=2)
                                mm_dr(ops[0:qp, :],
                                      [expT[:, 2 * j:2 * j + 2, qq0:qq0 + qp]
                                       for j in range(2)],
                                      [vsb[s][:, 2 * j:2 * j + 2,
                                              h * HS:h * HS + 65]
                                       for j in range(2)],
                                      tail=(expT[0:104, 4, qq0:qq0 + qp],
                                            vsb[s][0:104, 4, h * HS:h * HS + 65]))
                                rc = era.tile([128, 1], f32, name="rc", tag="rc",
                                              bufs=4)
                                nc.vector.reciprocal(rc[0:qp], ops[0:qp, 64:65])
                                nc.vector.tensor_scalar(
                                    osb[s][0:qp, qt, h * 64:(h + 1) * 64],
                                    ops[0:qp, 0:64], rc[0:qp], 1.0 / 16.0,
                                    Op.mult, Op.mult)

                        # ---- proj + residual for sample s (fp8 DoubleRow) ----
                        if LVL < 4:
                            continue
                        oT = era.tile([128, KC, 640], fp8, name=f"oT{s}",
                                      tag="xT", bufs=2)
                        for ti, (t0, tp) in enumerate(TOK_TILES):
                            for f in range(KC):
                                transpose_block(
                                    osb[s][:, ti, f * 128:(f + 1) * 128],
                                    oT[:, f, t0:t0 + 128],
                                    engines=[nc.scalar, nc.vector],
                                    scale=1.0 / 8.0)
                        x2[s] = era.tile([128, NT, C], bf16, name=f"x2_{s}",
                                         tag="x2", bufs=2)
                        for ti, (t0, tp) in enumerate(TOK_TILES):
                            xres = pp.tile([128, C], bf16, name="xres", tag="xres",
                                           bufs=2)
                            late_dma(xres[0:tp], xb_d[s, t0:t0 + tp, :])
                            for (n0, nn) in C_CHUNKS:
                                ps = ps_tile("ps_pj", nn)
                                mm_dr(ps[0:tp, 0:nn],
                                      [oT[:, 2 * k:2 * k + 2, t0:t0 + tp]
                                       for k in range(KP)],
                                      [wpj[:, 2 * k:2 * k + 2, n0:n0 + nn]
                                       for k in range(KP)])
                                nc.vector.tensor_add(
                                    x2[s][0:tp, ti, n0:n0 + nn],
                                    ps[0:tp, 0:nn], xres[0:tp, n0:n0 + nn])
                        # ---- repack x2[s] -> text [80, C] + img rows ----
                        nc.sync.dma_start(x2rep_txt[40 * s:40 * s + 40, :],
                                          x2[s][0:40, 0, :])
                        g = 576 * s
                        for kt, (t0, tp) in enumerate(TOK_TILES):
                            p0 = 40 if kt == 0 else 0
                            length = tp - p0
                            src_off = p0
                            while length > 0:
                                j, dp = g // 128, g % 128
                                piece = min(length, 128 - dp)
                                nc.sync.dma_start(
                                    x2rep_img[dp:dp + piece, j, :],
                                    x2[s][src_off:src_off + piece, kt, :])
                                g += piece
                                src_off += piece
                                length -= piece

            # ================= FFN era =================
            if LVL >= 5:
                with tc.tile_pool(name="ffn", bufs=1) as fp:
                    w2i = fp.tile([128, KF, C], fp8, name="w2i")
                    nc.sync.dma_start(w2i[:, 0:12, :], w2i_d[:, 0:12, :])
                    nc.sync.dma_start(w2i[:, 12:24, :], w2i_d[:, 12:24, :])
                    # LN2 + transpose
                    ztT = fp.tile([128, KC, 128], fp8, name="ztT")
                    xh2 = fp.tile([128, C], bf16, name="xh2", tag="xh2", bufs=2)
                    nc.vector.memset(xh2[64:128, :], 0.0)
                    layer_norm(fp, x2rep_txt[0:TXTTOK], TXTTOK, xh2[0:TXTTOK])
                    for f in range(KC):
                        transpose_block(xh2[:, f * 128:(f + 1) * 128],
                                        ztT[:, f, 0:128])
                    ziT = fp.tile([128, KC, IMGTOK], fp8, name="ziT")
                    for j in range(9):
                        xh2 = fp.tile([128, C], bf16, name="xh2", tag="xh2", bufs=2)
                        layer_norm(fp, x2rep_img[:, j, :], 128, xh2[:])
                        for f in range(KC):
                            transpose_block(xh2[:, f * 128:(f + 1) * 128],
                                            ziT[:, f, j * 128:(j + 1) * 128],
                                            engines=[nc.vector, nc.vector,
                                                     nc.scalar])
                    # Pre-add the (gamma_2-folded) fc2 biases into the residual so
                    # each fc2 eviction is a single tensor_add. In-place; Tile
                    # orders these after the LN2 reads above.
                    nc.gpsimd.tensor_add(x2rep_txt[0:TXTTOK, :], x2rep_txt[0:TXTTOK, :],
                                         b2t[0:TXTTOK, :])
                    for j in range(9):
                        nc.gpsimd.tensor_add(x2rep_img[:, j, :], x2rep_img[:, j, :],
                                             b2i[:, :])

                    # ---- img FFN (resident weights, 3 token chunks) ----
                    for c in range(3):
                        q0 = c * IMG_CHUNK
                        hgi = fp.tile([128, KF, IMG_CHUNK], fp8, name="hgi",
                                      tag="hgi", bufs=2)
                        for m in range(KF):
                            ps = ps_tile("ps_f1i", 512)
                            mm_dr(ps[:, 0:IMG_CHUNK],
                                  [w1i[:, 2 * k:2 * k + 2,
                                       m * 128:(m + 1) * 128]
                                   for k in range(KP)],
                                  [ziT[:, 2 * k:2 * k + 2, q0:q0 + IMG_CHUNK]
                                   for k in range(KP)])
                            nc.scalar.activation(hgi[:, m, :], ps[:, 0:IMG_CHUNK],
                                                 Af.Gelu, bias=b1i[:, m:m + 1],
                                                 scale=1.0 / 16.0)
                        for mt in range(3):
                            j = 3 * c + mt
                            # fc2 accumulators live on the "sc" banks (idle
                            # outside attention) so fc1's "big" ring flows.
                            ps0 = psum.tile([128, NPAD], f32, name="ps_f2i0",
                                            tag="sc", bufs=2)
                            ps1 = psum.tile([128, NPAD], f32, name="ps_f2i1",
                                            tag="sc", bufs=2)
                            mm_dr(ps0[:, 0:512],
                                  [hgi[:, 2 * k:2 * k + 2,
                                       mt * 128:(mt + 1) * 128]
                                   for k in range(KFP)],
                                  [w2i[:, 2 * k:2 * k + 2, 0:512]
                                   for k in range(KFP)])
                            mm_dr(ps1[:, 0:256],
                                  [hgi[:, 2 * k:2 * k + 2,
                                       mt * 128:(mt + 1) * 128]
                                   for k in range(KFP)],
                                  [w2i[:, 2 * k:2 * k + 2, 512:768]
                                   for k in range(KFP)])
                            ot = fp.tile([128, C], f32, name="ot", tag="ost", bufs=3)
                            for (n0, nn), ps in zip(C_CHUNKS, [ps0, ps1]):
                                nc.vector.tensor_add(ot[:, n0:n0 + nn], ps[:, 0:nn],
                                                     x2rep_img[:, j, n0:n0 + nn])
                            # DMA out: global img row g = 128*j -> (b, 40 + g%576)
                            g0 = 128 * j
                            p = 0
                            while p < 128:
                                g = g0 + p
                                b = g // IMG
                                piece = min(128 - p, IMG * (b + 1) - g)
                                late_dma(
                                    out_d[b, TXT + g - b * IMG:
                                          TXT + g - b * IMG + piece, :],
                                    ot[p:p + piece, :])
                                p += piece

                    # ---- text FFN (streamed weights, fp8 DoubleRow) ----
                    with tc.tile_pool(name="wtxt", bufs=1) as wt:
                        hgt = fp.tile([128, KF, TXTTOK], fp8, name="hgt")
                        for mc in range(8):
                            w1tc = wt.tile([128, 3, KC, 128], fp8,
                                           name="w1tc", tag="w1tc", bufs=2)
                            nc.sync.dma_start(
                                w1tc[:],
                                w1t_d[3 * mc:3 * mc + 3].rearrange(
                                    "m p k n -> p m k n"))
                            for ml in range(3):
                                m = 3 * mc + ml
                                ps = ps_tile("ps_f1t", 512)
                                mm_dr(ps[:, 0:TXTTOK],
                                      [w1tc[:, ml, 2 * k:2 * k + 2, :]
                                       for k in range(KP)],
                                      [ztT[:, 2 * k:2 * k + 2, 0:TXTTOK]
                                       for k in range(KP)])
                                nc.scalar.activation(
                                    hgt[:, m, 0:TXTTOK], ps[:, 0:TXTTOK],
                                    Af.Gelu, bias=b1t[:, m:m + 1],
                                    scale=1.0 / 16.0)
                        ps0 = psum.tile([128, NPAD], f32, name="ps_f2t0",
                                        tag="sc", bufs=2)
                        ps1 = psum.tile([128, NPAD], f32, name="ps_f2t1",
                                        tag="sc", bufs=2)
                        for kc4 in range(6):
                            w2tc = wt.tile([128, 4, C], fp8, name="w2tc",
                                           tag="w2tc", bufs=2)
                            nc.sync.dma_start(
                                w2tc[:],
                                w2t_d[4 * kc4:4 * kc4 + 4].rearrange(
                                    "k p n -> p k n"))
                            for kl in range(2):
                                kp = 2 * kc4 + kl     # global k-pair index
                                nc.tensor.matmul(
                                    ps0[0:TXTTOK, 0:512],
                                    hgt[:, 2 * kp:2 * kp + 2, 0:TXTTOK],
                                    w2tc[:, 2 * kl:2 * kl + 2, 0:512],
                                    start=(kp == 0), stop=(kp == KFP - 1),
                                    perf_mode=DR)
                                nc.tensor.matmul(
                                    ps1[0:TXTTOK, 0:256],
                                    hgt[:, 2 * kp:2 * kp + 2, 0:TXTTOK],
                                    w2tc[:, 2 * kl:2 * kl + 2, 512:768],
                                    start=(kp == 0), stop=(kp == KFP - 1),
                                    perf_mode=DR)
                        ot = fp.tile([128, C], f32, name="ot", tag="ost", bufs=3)
                        for (n0, nn), ps in zip(C_CHUNKS, [ps0, ps1]):
                            nc.vector.tensor_add(ot[0:TXTTOK, n0:n0 + nn],
                                                 ps[0:TXTTOK, 0:nn],
                                                 x2rep_txt[0:TXTTOK, n0:n0 + nn])
                        for s in range(S):
                            late_dma(out_d[s, 0:TXT, :],
                                     ot[40 * s:40 * s + 40, :])
            fw_ctx.__exit__(None, None, None)

    nc.compile()
    return nc


_CACHE = {}


def _get_program():
    if "nc" not in _CACHE:
        _CACHE["nc"] = build_program()
    return _CACHE["nc"]


def run(inputs, trace=False):
    from concourse.bass_utils import run_bass_kernel_spmd
    shared, per_core = host_prep(inputs)
    nc = _get_program()
    in_maps = [{**shared, **pc} for pc in per_core]
    res = run_bass_kernel_spmd(nc, in_maps, core_ids=list(range(NCORES)),
                               trace=trace)
    out = np.concatenate([res.results[i]["out"] for i in range(NCORES)],
                         axis=0).astype(np.float32)
    return out, res


def kernel(**inputs):
    out, _ = run(inputs, trace=False)
    return out
